# revision 23
# baseline (speedup 1.0000x reference)
"""GQA dense-transformer kernel for 8 Trainium2 NeuronCores.

Problem (hardcoded): B=2, S=2048, D=2048, kv_heads=16, groups G=4, HPG=4,
HD=128.  reference:
    qkv = x @ Wqkv + bqkv ; q,k,v = split(qkv)
    q = einsum('bsghd,gde->bsghe', q, Wq) + bq   (per-group shared proj)
    v = einsum('bsghd,gde->bsghe', v, Wv) + bv
    scores = einsum('bqghd,bkghd->bghqk', q, k) / sqrt(HD)
    attn = softmax(scores) * attn_mask           (mask == ones at grading)
    out = einsum('bghqk,bkghd->bqghd', attn, v)  -> [B,S,D]

Sharding: core c = b*4 + g handles (batch b, group g): it computes the
512 output columns [g*512,(g+1)*512) of out[b].

Per-core device program (bf16 matmuls, fp32 PSUM):
  The per-group query projection Wq (and the attention scale) is folded
  into Wqkv on the HOST: W_q2'[:,hs] = Wqkv_q[:,hs] @ (Wq[g]*SCALE), so
  the device computes q2^T and k^T directly in one projection pass.
  All inputs are RELAID OUT partition-major on the host so every DMA
  descriptor covers a 4-16KB contiguous run (the dominant startup cost
  is per-descriptor overhead, not bytes).
  phase 1: chunk 0 (s in [0,512)) runs K-OUTER in groups of 4 k-blocks
           with 8 live PSUM banks (m=0..7 = q2',k), so the PE starts
           after ~1.5MB of DMA and then tracks the DMA stream; v
           (m=8..11) + chunks 1..3 run m-outer.  v2 natural blocks via
           lhsT=v1^T slices @ wv.  q2/k evacuate on ACT (bias add),
           v1/v2 on DVE.
  phase 2: software-pipelined across heads: for each j, scores(h) j
           (PE->PSUM, exp on ACT) is interleaved with PV(h-1) j
           (j-MAJOR accumulation into 4 simultaneous po banks), so the
           PE always has dense work while ACT drains the exps.
           Denominators per head via concurrent M=1 ones-matmuls in 4
           PE col-groups + one fp32r selector matmul (psum banks shared
           with po via tag rotation).  Output is UNNORMALIZED out^T +
           denominators; softmax division + v-path bias happen on host.
"""
import sys
import numpy as np

sys.path.insert(0, "/opt/trn_rl_repo")
import ml_dtypes  # noqa: E402

B, S, D = 2, 2048, 2048
G, HPG, HD = 4, 4, 128
GC = HPG * HD            # 512 columns per group
SCALE = HD ** -0.5
P = 128
KB = D // P              # 16 contraction blocks
SB = S // P              # 16 sk blocks
NCORES = 8

_CACHE: dict = {}


def _build_program():
    import concourse.tile_sem_assignment as tsa
    # Walrus caps sync waits per instruction.  Tile's vector clock emits
    # transitive waits, so cap HWDGE sems at 1; _split_excess_waits breaks
    # any remaining multi-wait instruction into standalone EventSemaphore
    # CTRLs on the same engine.
    tsa.NUM_HWDGE_SEMS = 1

    import concourse.bass as bass
    import concourse.tile as tile
    from concourse import mybir
    from contextlib import ExitStack

    bf16 = mybir.dt.bfloat16
    f32 = mybir.dt.float32

    nc = bass.Bass(trn_type="TRN2")
    SCH = 512                 # s-chunk width for projection phase
    NCH = S // SCH            # 4 chunks
    QCH = 1024                # sq chunk width for scores/exp
    # partition-major input layouts (see kernel() for the host side)
    xt_d = nc.dram_tensor("xt", [NCH, P, KB, SCH], bf16, kind="ExternalInput")
    wqk_d = nc.dram_tensor("wqk", [P, KB, 2 * GC], bf16, kind="ExternalInput")
    wv1_d = nc.dram_tensor("wv1", [P, KB, GC], bf16, kind="ExternalInput")
    b1_d = nc.dram_tensor("b1", [P, 8], f32, kind="ExternalInput")
    wv_d = nc.dram_tensor("wv", [HD, HD], bf16, kind="ExternalInput")
    onesc_d = nc.dram_tensor("onesc", [P, 1], bf16, kind="ExternalInput")
    sel_d = nc.dram_tensor("sel", [P, 1], f32, kind="ExternalInput")
    out_d = nc.dram_tensor("out", [GC, S], f32, kind="ExternalOutput")
    den_d = nc.dram_tensor("den", [HPG, S], f32, kind="ExternalOutput")

    Exp = mybir.ActivationFunctionType.Exp
    Ident = mybir.ActivationFunctionType.Identity

    with tile.TileContext(nc) as tc:
        with ExitStack() as octx:
            # ---- persistent tiles ----
            persist = octx.enter_context(tc.tile_pool(name="persist", bufs=1))
            k_sb = persist.tile([P, HPG, S], bf16)       # k^T per head
            q2_sb = persist.tile([P, HPG, S], bf16)      # q2^T per head
            v2_sb = persist.tile([P, HPG, SB, HD], bf16)  # v2 natural blocks
            wv_sb = persist.tile([HD, HD], bf16)
            b1_sb = persist.tile([P, 8], f32)
            ones_sb = persist.tile([P, 1], bf16)
            sel_f = persist.tile([P, 1], f32)
            sel_sb = persist.tile([P, 1], mybir.dt.float32r)
            nc.sync.dma_start(wv_sb[:], wv_d[:])
            nc.sync.dma_start(b1_sb[:], b1_d[:])
            nc.sync.dma_start(ones_sb[:], onesc_d[:])
            nc.sync.dma_start(sel_f[:], sel_d[:])
            nc.vector.tensor_copy(sel_sb[:], sel_f[:])

            # ---------------- phase 1: projections ----------------
            with ExitStack() as ctx:
                wpool = ctx.enter_context(tc.tile_pool(name="w1", bufs=1))
                xpool = ctx.enter_context(tc.tile_pool(name="xT", bufs=3))
                tpool = ctx.enter_context(tc.tile_pool(name="tmp", bufs=4))
                wqk_sb = wpool.tile([P, KB, 2 * GC], bf16)
                wv1_sb = wpool.tile([P, KB, GC], bf16)
                xT0 = wpool.tile([P, KB, SCH], bf16)

                def wslice(m, k):
                    if m < 8:
                        return wqk_sb[:, k, m * P:(m + 1) * P]
                    return wv1_sb[:, k, (m - 8) * P:(m - 7) * P]

                # stage A: chunk 0, m=0..7 (q2', k), K-OUTER in groups of
                # 4 k-blocks with 8 live PSUM banks.  Each group's DMA is
                # one 128-descriptor transfer with 4-8KB contiguous runs.
                with ExitStack() as actx:
                    p0 = actx.enter_context(
                        tc.tile_pool(name="p0", bufs=1, space="PSUM"))
                    psA = [p0.tile([P, SCH], f32, tag=f"m{m}", name=f"psA{m}")
                           for m in range(8)]
                    # split every group DMA into partition-quarters so each
                    # ~0.5MB group fans out over 8 DMA engines in parallel
                    # (single-engine throughput is only ~40GB/s).
                    for g4 in range(4):
                        ks = slice(4 * g4, 4 * g4 + 4)
                        for q in range(4):
                            ps_ = slice(32 * q, 32 * q + 32)
                            nc.sync.dma_start(wqk_sb[ps_, ks],
                                              wqk_d[ps_, ks])
                            nc.sync.dma_start(xT0[ps_, ks],
                                              xt_d[0, ps_, ks])
                    for q in range(4):
                        ps_ = slice(32 * q, 32 * q + 32)
                        nc.sync.dma_start(wv1_sb[ps_], wv1_d[ps_])
                    for k in range(KB):
                        for m in range(8):
                            nc.tensor.matmul(
                                psA[m][:], wslice(m, k), xT0[:, k],
                                start=(k == 0), stop=(k == KB - 1))
                    for m in range(8):
                        dst = q2_sb if m < 4 else k_sb
                        nc.scalar.activation(
                            dst[:, m % 4, 0:SCH], psA[m][:],
                            Ident, bias=b1_sb[:, m:m + 1])

                # stage B: v for chunk 0, then all m for chunks 1..3
                with ExitStack() as bctx:
                    pp = bctx.enter_context(
                        tc.tile_pool(name="pp", bufs=3, space="PSUM"))
                    pv = bctx.enter_context(
                        tc.tile_pool(name="pv", bufs=2, space="PSUM"))

                    def vblock(ps, h, c):
                        """v1 psum chunk -> v1 tmp -> v2 natural blocks."""
                        v1 = tpool.tile([P, SCH], bf16, tag="v1")
                        nc.vector.tensor_copy(v1[:], ps[:])
                        for sb in range(SCH // P):
                            ps3 = pv.tile([P, HD], f32)
                            nc.tensor.matmul(
                                ps3[:], v1[:, sb * P:(sb + 1) * P],
                                wv_sb[:], start=True, stop=True)
                            nc.vector.tensor_copy(
                                v2_sb[:, h, c * (SCH // P) + sb, :], ps3[:])

                    for m in range(8, 12):
                        ps = pp.tile([P, SCH], f32)
                        for k in range(KB):
                            nc.tensor.matmul(
                                ps[:], wslice(m, k), xT0[:, k],
                                start=(k == 0), stop=(k == KB - 1))
                        vblock(ps, m - 8, 0)

                    for c in range(1, NCH):
                        xT = xpool.tile([P, KB, SCH], bf16, tag="xT")
                        for q in range(4):
                            ps_ = slice(32 * q, 32 * q + 32)
                            nc.sync.dma_start(xT[ps_], xt_d[c, ps_])
                        for m in range(12):
                            ps = pp.tile([P, SCH], f32)
                            for k in range(KB):
                                nc.tensor.matmul(
                                    ps[:], wslice(m, k), xT[:, k],
                                    start=(k == 0), stop=(k == KB - 1))
                            if m < 8:
                                dst = q2_sb if m < 4 else k_sb
                                nc.scalar.activation(
                                    dst[:, m % 4, c * SCH:(c + 1) * SCH],
                                    ps[:], Ident, bias=b1_sb[:, m:m + 1])
                            else:
                                vblock(ps, m - 8, c)

            # ---------------- phase 2: attention ----------------
            # software pipeline: per j, scores(h) j interleaves with the
            # j-MAJOR PV of the SAME head at lag 1 (PV j-1 runs while
            # EXP j drains), so the PE tracks ACT with no dense-PV tail.
            with ExitStack() as ctx:
                ppool = ctx.enter_context(tc.tile_pool(name="P", bufs=24))
                opool = ctx.enter_context(tc.tile_pool(name="osb", bufs=4))
                dpool = ctx.enter_context(tc.tile_pool(name="dsb", bufs=4))
                sps = ctx.enter_context(
                    tc.tile_pool(name="sps", bufs=2, space="PSUM"))
                ops = ctx.enter_context(
                    tc.tile_pool(name="ops", bufs=1, space="PSUM"))

                def scores_j(h, j, Pj):
                    for qc in range(S // QCH):
                        ss = sps.tile([P, QCH], f32, name="ss")
                        for half in range(QCH // 512):
                            off = qc * QCH + half * 512
                            nc.tensor.matmul(
                                ss[:, half * 512:(half + 1) * 512],
                                k_sb[:, h, j * P:(j + 1) * P],
                                q2_sb[:, h, off:off + 512],
                                start=True, stop=True)
                        nc.scalar.activation(
                            Pj[:, qc * QCH:(qc + 1) * QCH], ss[:], Exp)

                def pv_den(h, Ph, po):
                    """emit PV epilogue + denominators for head h."""
                    for qc in range(4):
                        sl = slice(qc * 512, (qc + 1) * 512)
                        osb = opool.tile([P, 512], f32, tag="o", name="osb")
                        nc.vector.tensor_copy(osb[:], po[qc][:])
                        for q in range(4):
                            nc.sync.dma_start(
                                out_d[h * P + 32 * q:h * P + 32 * q + 32, sl],
                                osb[32 * q:32 * q + 32, :])
                        # denominators: 4 concurrent M=1 ones-matmuls in 4
                        # col-groups x 4 rounds, then fp32r selector matmul.
                        pd = ops.tile([P, 512], f32, tag=f"po{qc}", name="pd")
                        for r in range(4):
                            for jj in range(4):
                                j = r * 4 + jj
                                nc.tensor.matmul(
                                    pd[32 * jj:32 * jj + 1, :],
                                    ones_sb[:, 0:1], Ph[j][:, sl],
                                    start=(r == 0), stop=(r == 3),
                                    tile_position=(0, 32 * jj))
                        parts = dpool.tile([97, 512], mybir.dt.float32r,
                                           tag="dp", name="parts")
                        nc.vector.tensor_copy(parts[:], pd[0:97, :])
                        pd2 = ops.tile([P, 512], f32, tag=f"po{qc}",
                                       name="pd2")
                        nc.tensor.matmul(pd2[0:1, :], sel_sb[0:97, 0:1],
                                         parts[:], start=True, stop=True)
                        dsb = dpool.tile([1, 512], f32, tag="d", name="dsb")
                        nc.vector.tensor_copy(dsb[:], pd2[0:1, :])
                        nc.sync.dma_start(den_d[h:h + 1, sl], dsb[:])

                # Same-head PV at lag 3: PV(h) j runs while ACT drains
                # EXP(h) j+1..j+3, so the PE tracks ACT with only a ~3-j
                # PV tail per head.  pv_den(h-1) is emitted at j==3 of
                # head h (its exps finished a full block ago, so it never
                # stalls), IMMEDIATELY BEFORE po(h)'s allocation: the
                # shared po/pd psum-bank WAR chain then follows emission
                # order (osb(h-1) -> pd(h-1) -> pd2(h-1) -> po(h)), so no
                # deadlock.
                LAG = 3

                def pv_j(h, j, po, Ph):
                    for qc in range(4):
                        nc.tensor.matmul(
                            po[qc][:], v2_sb[:, h, j, :],
                            Ph[j][:, qc * 512:(qc + 1) * 512],
                            start=(j == 0), stop=(j == SB - 1))

                prev = None
                for h in range(HPG):
                    Ph = []
                    po = None
                    for j in range(SB):
                        Pj = ppool.tile([P, S], bf16, tag="P", name="Pj")
                        scores_j(h, j, Pj)
                        Ph.append(Pj)
                        if j == LAG:
                            if prev is not None:
                                pv_den(prev[0], prev[1], prev[2])
                            po = [ops.tile([P, 512], f32, tag=f"po{qc}",
                                           name=f"po{qc}") for qc in range(4)]
                        if j >= LAG:
                            pv_j(h, j - LAG, po, Ph)
                    for j in range(SB - LAG, SB):
                        pv_j(h, j, po, Ph)
                    prev = (h, Ph, po)
                pv_den(prev[0], prev[1], prev[2])

    _split_excess_waits(nc, mybir)
    return nc


def _split_excess_waits(nc, mybir):
    """Each TPB instruction has ONE wait slot (NEURON_ISA_TPB_EVENTS); walrus
    refuses instructions with more sync waits.  Tile attaches the full
    vector-clock wait list to instructions, so split all but one wait out
    into standalone EventSemaphore (CTRL) instructions on the same engine,
    placed immediately before.  Semantics are identical: all waits must be
    satisfied before the instruction executes."""
    import copy
    template = None
    for blk in nc.m.functions[0].blocks:
        for inst in blk.instructions:
            if isinstance(inst, mybir.InstEventSemaphore):
                template = inst
                break
        if template is not None:
            break
    assert template is not None, "no EventSemaphore template found"
    uid = [0]
    for fn in nc.m.functions:
        for blk in fn.blocks:
            out = []
            for inst in blk.instructions:
                si = inst.sync_info
                if si is not None and len(si.on_wait) > 1:
                    waits = list(si.on_wait)
                    for w in waits[:-1]:
                        ev = copy.deepcopy(template)
                        ev.name = f"swsplit-{uid[0]}"
                        uid[0] += 1
                        ev.engine = inst.engine
                        ev.sync_info = mybir.SyncInfo(on_wait=[w], on_update=[])
                        out.append(ev)
                    si.on_wait = waits[-1:]
                    inst.sync_info = si
                out.append(inst)
            blk.instructions[:] = out
    return nc


def _numpy_fallback(x, attn_mask, Wqkv, bqkv, Wq, bq, Wv, bv):
    x = np.asarray(x, np.float32)
    qkv = x @ np.asarray(Wqkv, np.float32) + np.asarray(bqkv, np.float32)
    q, k, v = np.split(qkv, 3, axis=-1)
    q = q.reshape(B, S, G, HPG, HD)
    k = k.reshape(B, S, G, HPG, HD)
    v = v.reshape(B, S, G, HPG, HD)
    q = np.einsum('bsghd,gde->bsghe', q, np.asarray(Wq, np.float32)) \
        + np.asarray(bq, np.float32)[None, None, :, None, :]
    v = np.einsum('bsghd,gde->bsghe', v, np.asarray(Wv, np.float32)) \
        + np.asarray(bv, np.float32)[None, None, :, None, :]
    out = np.empty((B, S, G, HPG, HD), np.float32)
    for b in range(B):
        for g in range(G):
            for hh in range(HPG):
                s = (q[b, :, g, hh] @ k[b, :, g, hh].T) * SCALE
                s = s - s.max(axis=-1, keepdims=True)
                p = np.exp(s)
                p /= p.sum(axis=-1, keepdims=True)
                p = p * np.asarray(attn_mask, np.float32)
                out[b, :, g, hh] = p @ v[b, :, g, hh]
    return out.reshape(B, S, D)


def kernel(x, attn_mask, Wqkv, bqkv, Wq, bq, Wv, bv):
    x = np.asarray(x)
    attn_mask = np.asarray(attn_mask)
    Wqkv = np.asarray(Wqkv)
    bqkv = np.asarray(bqkv)
    Wq = np.asarray(Wq)
    bq = np.asarray(bq)
    Wv = np.asarray(Wv)
    bv = np.asarray(bv)

    if not np.all(attn_mask == 1.0):
        # general (non-ones) post-softmax mask: correct but slow host path
        return _numpy_fallback(x, attn_mask, Wqkv, bqkv, Wq, bq, Wv, bv)

    if "nc" not in _CACHE:
        _CACHE["nc"] = _build_program()
    nc = _CACHE["nc"]
    from concourse.bass_utils import run_bass_kernel_spmd

    bf = ml_dtypes.bfloat16
    sel_np = np.zeros((P, 1), np.float32)
    sel_np[0::32] = 1.0
    in_maps = []
    # xt layout [chunk, p, ko, s']: xt[c,p,ko,s'] = x[b][c*512+s', ko*128+p]
    x_bf = []
    for b in range(B):
        xT = np.asarray(x[b], np.float32).T.astype(bf)      # [D, S]
        x_bf.append(np.ascontiguousarray(
            xT.reshape(KB, P, NCORES // 2, 512).transpose(2, 1, 0, 3)))
    Wq32 = np.asarray(Wq, np.float32)
    Wv32 = np.asarray(Wv, np.float32)
    host_bias = []

    def pmajor(w):
        """[D, N] -> [P, KB, N] with [p, ko, n] = w[ko*128+p, n]"""
        return np.ascontiguousarray(
            w.reshape(KB, P, w.shape[1]).transpose(1, 0, 2))

    for c in range(NCORES):
        b, g = divmod(c, G)
        cols = slice(g * GC, (g + 1) * GC)
        wq_c = Wqkv[:, 0 * D:1 * D][:, cols].astype(np.float32)
        wk_c = Wqkv[:, 1 * D:2 * D][:, cols]
        wv_c = Wqkv[:, 2 * D:3 * D][:, cols]
        # fold the per-group query projection + attention scale on host:
        wqs = Wq32[g] * SCALE
        wq_fold = (wq_c.reshape(D, HPG, HD) @ wqs[None]).reshape(D, GC)
        wqk = np.concatenate([wq_fold.astype(bf), np.asarray(wk_c, bf)],
                             axis=1)
        bq1 = bqkv[0 * D:1 * D][cols].astype(np.float32)
        bk1 = bqkv[1 * D:2 * D][cols].astype(np.float32)
        bv1 = bqkv[2 * D:3 * D][cols].astype(np.float32)
        bq2 = (bq1.reshape(HPG, HD) @ wqs
               + np.asarray(bq, np.float32)[g] * SCALE).reshape(GC)
        b1cat = np.concatenate([bq2, bk1]).astype(np.float32)
        host_bias.append(
            (bv1.reshape(HPG, HD) @ Wv32[g]
             + np.asarray(bv, np.float32)[g][None, :]).reshape(GC))
        in_maps.append({
            "xt": x_bf[b],
            "wqk": pmajor(wqk),
            "wv1": pmajor(np.asarray(wv_c, bf)),
            "b1": np.ascontiguousarray(b1cat.reshape(8, P).T),
            "wv": np.ascontiguousarray(Wv32[g].astype(bf)),
            "onesc": np.ones((P, 1), bf),
            "sel": sel_np,
        })

    res = run_bass_kernel_spmd(nc, in_maps, list(range(NCORES)),
                               **_CACHE.get("run_kwargs", {}))
    _CACHE["last_results"] = res

    out = np.empty((B, S, D), np.float32)
    for c in range(NCORES):
        b, g = divmod(c, G)
        o = res.results[c]["out"]          # [GC, S] unnormalized out^T
        den = res.results[c]["den"]        # [HPG, S]
        o = o / np.repeat(den, HD, axis=0)  # normalize rows h*128+e by den[h]
        o = o + host_bias[c][:, None]
        out[b, :, g * GC:(g + 1) * GC] = o.T
    return out


# revision 27
# speedup vs baseline: 1.2793x; 1.2793x over previous
"""GQA dense-transformer kernel for 8 Trainium2 NeuronCores.

Problem (hardcoded): B=2, S=2048, D=2048, kv_heads=16, groups G=4, HPG=4,
HD=128.  reference:
    qkv = x @ Wqkv + bqkv ; q,k,v = split(qkv)
    q = einsum('bsghd,gde->bsghe', q, Wq) + bq   (per-group shared proj)
    v = einsum('bsghd,gde->bsghe', v, Wv) + bv
    scores = einsum('bqghd,bkghd->bghqk', q, k) / sqrt(HD)
    attn = softmax(scores) * attn_mask           (mask == ones at grading)
    out = einsum('bghqk,bkghd->bqghd', attn, v)  -> [B,S,D]

Sharding: core c = b*4 + g handles (batch b, group g): it computes the
512 output columns [g*512,(g+1)*512) of out[b].

Per-core device program (bf16 matmuls, fp32 PSUM):
  The per-group query projection Wq (and the attention scale) is folded
  into Wqkv on the HOST: W_q2'[:,hs] = Wqkv_q[:,hs] @ (Wq[g]*SCALE), so
  the device computes q2^T and k^T directly in one projection pass.
  All inputs are RELAID OUT partition-major on the host so every DMA
  descriptor covers a 4-16KB contiguous run (the dominant startup cost
  is per-descriptor overhead, not bytes).
  phase 1: chunk 0 (s in [0,512)) runs K-OUTER in groups of 4 k-blocks
           with 8 live PSUM banks (m=0..7 = q2',k), so the PE starts
           after ~1.5MB of DMA and then tracks the DMA stream; v
           (m=8..11) + chunks 1..3 run m-outer.  v2 natural blocks via
           lhsT=v1^T slices @ wv.  q2/k evacuate on ACT (bias add),
           v1/v2 on DVE.
  phase 2: software-pipelined across heads: for each j, scores(h) j
           (PE->PSUM, exp on ACT) is interleaved with PV(h-1) j
           (j-MAJOR accumulation into 4 simultaneous po banks), so the
           PE always has dense work while ACT drains the exps.
           Denominators per head via concurrent M=1 ones-matmuls in 4
           PE col-groups + one fp32r selector matmul (psum banks shared
           with po via tag rotation).  Output is UNNORMALIZED out^T +
           denominators; softmax division + v-path bias happen on host.
"""
import sys
import numpy as np

sys.path.insert(0, "/opt/trn_rl_repo")
import ml_dtypes  # noqa: E402

B, S, D = 2, 2048, 2048
G, HPG, HD = 4, 4, 128
GC = HPG * HD            # 512 columns per group
SCALE = HD ** -0.5
P = 128
KB = D // P              # 16 contraction blocks
SB = S // P              # 16 sk blocks
NCORES = 8

_CACHE: dict = {}


def _build_program():
    import concourse.tile_sem_assignment as tsa
    # Walrus caps sync waits per instruction.  Tile's vector clock emits
    # transitive waits, so cap HWDGE sems at 1; _split_excess_waits breaks
    # any remaining multi-wait instruction into standalone EventSemaphore
    # CTRLs on the same engine.
    tsa.NUM_HWDGE_SEMS = 1

    import concourse.bass as bass
    import concourse.tile as tile
    from concourse import mybir
    from contextlib import ExitStack

    bf16 = mybir.dt.bfloat16
    f32 = mybir.dt.float32

    nc = bass.Bass(trn_type="TRN2")
    SCH = 512                 # s-chunk width for projection phase
    NCH = S // SCH            # 4 chunks
    QCH = 1024                # sq chunk width for scores/exp
    # partition-major input layouts (see kernel() for the host side)
    xt_d = nc.dram_tensor("xt", [NCH, P, KB, SCH], bf16, kind="ExternalInput")
    wqk_d = nc.dram_tensor("wqk", [P, KB, 2 * GC], bf16, kind="ExternalInput")
    wv1_d = nc.dram_tensor("wv1", [P, KB, GC], bf16, kind="ExternalInput")
    b1_d = nc.dram_tensor("b1", [P, 8], f32, kind="ExternalInput")
    wv_d = nc.dram_tensor("wv", [HD, HD], bf16, kind="ExternalInput")
    onesc_d = nc.dram_tensor("onesc", [P, 1], bf16, kind="ExternalInput")
    sel_d = nc.dram_tensor("sel", [P, 1], f32, kind="ExternalInput")
    out_d = nc.dram_tensor("out", [GC, S], f32, kind="ExternalOutput")
    den_d = nc.dram_tensor("den", [HPG, S], f32, kind="ExternalOutput")

    Exp = mybir.ActivationFunctionType.Exp
    Ident = mybir.ActivationFunctionType.Identity

    with tile.TileContext(nc) as tc:
        with ExitStack() as octx:
            # ---- persistent tiles ----
            persist = octx.enter_context(tc.tile_pool(name="persist", bufs=1))
            k_sb = persist.tile([P, HPG, S], bf16)       # k^T per head
            q2_sb = persist.tile([P, HPG, S], bf16)      # q2^T per head
            v2_sb = persist.tile([P, HPG, SB, HD], bf16)  # v2 natural blocks
            wv_sb = persist.tile([HD, HD], bf16)
            b1_sb = persist.tile([P, 8], f32)
            ones_sb = persist.tile([P, 1], bf16)
            sel_f = persist.tile([P, 1], f32)
            sel_sb = persist.tile([P, 1], mybir.dt.float32r)
            nc.sync.dma_start(wv_sb[:], wv_d[:])
            nc.sync.dma_start(b1_sb[:], b1_d[:])
            nc.sync.dma_start(ones_sb[:], onesc_d[:])
            nc.sync.dma_start(sel_f[:], sel_d[:])
            nc.vector.tensor_copy(sel_sb[:], sel_f[:])

            # ---------------- phase 1: projections ----------------
            with ExitStack() as ctx:
                wpool = ctx.enter_context(tc.tile_pool(name="w1", bufs=1))
                xpool = ctx.enter_context(tc.tile_pool(name="xT", bufs=2))
                tpool = ctx.enter_context(tc.tile_pool(name="tmp", bufs=4))
                wqk_sb = wpool.tile([P, KB, 2 * GC], bf16)
                wv1_sb = wpool.tile([P, KB, GC], bf16)
                xT0 = wpool.tile([P, KB, SCH], bf16)

                def wslice(m, k):
                    if m < 8:
                        return wqk_sb[:, k, m * P:(m + 1) * P]
                    return wv1_sb[:, k, (m - 8) * P:(m - 7) * P]

                # stage A: chunk 0, m=0..7 (q2', k), K-OUTER in groups of
                # 4 k-blocks with 8 live PSUM banks.  Each group's DMA is
                # one 128-descriptor transfer with 4-8KB contiguous runs.
                with ExitStack() as actx:
                    p0 = actx.enter_context(
                        tc.tile_pool(name="p0", bufs=1, space="PSUM"))
                    psA = [p0.tile([P, SCH], f32, tag=f"m{m}", name=f"psA{m}")
                           for m in range(8)]
                    for g4 in range(4):
                        ks = slice(4 * g4, 4 * g4 + 4)
                        nc.sync.dma_start(wqk_sb[:, ks], wqk_d[:, ks])
                        nc.sync.dma_start(xT0[:, ks], xt_d[0, :, ks])
                    nc.sync.dma_start(wv1_sb[:], wv1_d[:])
                    for k in range(KB):
                        for m in range(8):
                            nc.tensor.matmul(
                                psA[m][:], wslice(m, k), xT0[:, k],
                                start=(k == 0), stop=(k == KB - 1))
                    for m in range(8):
                        dst = q2_sb if m < 4 else k_sb
                        nc.scalar.activation(
                            dst[:, m % 4, 0:SCH], psA[m][:],
                            Ident, bias=b1_sb[:, m:m + 1])

                # stage B: v for chunk 0, then all m for chunks 1..3
                with ExitStack() as bctx:
                    pp = bctx.enter_context(
                        tc.tile_pool(name="pp", bufs=3, space="PSUM"))
                    pv = bctx.enter_context(
                        tc.tile_pool(name="pv", bufs=2, space="PSUM"))

                    def vblock(ps, h, c):
                        """v1 psum chunk -> v1 tmp -> v2 natural blocks."""
                        v1 = tpool.tile([P, SCH], bf16, tag="v1")
                        nc.vector.tensor_copy(v1[:], ps[:])
                        for sb in range(SCH // P):
                            ps3 = pv.tile([P, HD], f32)
                            nc.tensor.matmul(
                                ps3[:], v1[:, sb * P:(sb + 1) * P],
                                wv_sb[:], start=True, stop=True)
                            nc.vector.tensor_copy(
                                v2_sb[:, h, c * (SCH // P) + sb, :], ps3[:])

                    for m in range(8, 12):
                        ps = pp.tile([P, SCH], f32)
                        for k in range(KB):
                            nc.tensor.matmul(
                                ps[:], wslice(m, k), xT0[:, k],
                                start=(k == 0), stop=(k == KB - 1))
                        vblock(ps, m - 8, 0)

                    for c in range(1, NCH):
                        xT = xpool.tile([P, KB, SCH], bf16, tag="xT")
                        nc.sync.dma_start(xT[:], xt_d[c])
                        for m in range(12):
                            ps = pp.tile([P, SCH], f32)
                            for k in range(KB):
                                nc.tensor.matmul(
                                    ps[:], wslice(m, k), xT[:, k],
                                    start=(k == 0), stop=(k == KB - 1))
                            if m < 8:
                                dst = q2_sb if m < 4 else k_sb
                                nc.scalar.activation(
                                    dst[:, m % 4, c * SCH:(c + 1) * SCH],
                                    ps[:], Ident, bias=b1_sb[:, m:m + 1])
                            else:
                                vblock(ps, m - 8, c)

            # ---------------- phase 2: attention ----------------
            # software pipeline: per j, scores(h) j interleaves with the
            # j-MAJOR PV of the SAME head at lag 1 (PV j-1 runs while
            # EXP j drains), so the PE tracks ACT with no dense-PV tail.
            with ExitStack() as ctx:
                ppool = ctx.enter_context(tc.tile_pool(name="P", bufs=24))
                opool = ctx.enter_context(tc.tile_pool(name="osb", bufs=4))
                dpool = ctx.enter_context(tc.tile_pool(name="dsb", bufs=4))
                sps = ctx.enter_context(
                    tc.tile_pool(name="sps", bufs=2, space="PSUM"))
                ops = ctx.enter_context(
                    tc.tile_pool(name="ops", bufs=1, space="PSUM"))

                def scores_j(h, j, Pj):
                    for qc in range(S // QCH):
                        ss = sps.tile([P, QCH], f32, name="ss")
                        for half in range(QCH // 512):
                            off = qc * QCH + half * 512
                            nc.tensor.matmul(
                                ss[:, half * 512:(half + 1) * 512],
                                k_sb[:, h, j * P:(j + 1) * P],
                                q2_sb[:, h, off:off + 512],
                                start=True, stop=True)
                        nc.scalar.activation(
                            Pj[:, qc * QCH:(qc + 1) * QCH], ss[:], Exp)

                def pv_den(h, Ph, po):
                    """emit PV epilogue + denominators for head h."""
                    for qc in range(4):
                        sl = slice(qc * 512, (qc + 1) * 512)
                        osb = opool.tile([P, 512], f32, tag="o", name="osb")
                        nc.vector.tensor_copy(osb[:], po[qc][:])
                        nc.sync.dma_start(out_d[h * P:(h + 1) * P, sl], osb[:])
                        # denominators: 4 concurrent M=1 ones-matmuls in 4
                        # col-groups x 4 rounds, then fp32r selector matmul.
                        pd = ops.tile([P, 512], f32, tag=f"po{qc}", name="pd")
                        for r in range(4):
                            for jj in range(4):
                                j = r * 4 + jj
                                nc.tensor.matmul(
                                    pd[32 * jj:32 * jj + 1, :],
                                    ones_sb[:, 0:1], Ph[j][:, sl],
                                    start=(r == 0), stop=(r == 3),
                                    tile_position=(0, 32 * jj))
                        parts = dpool.tile([97, 512], mybir.dt.float32r,
                                           tag="dp", name="parts")
                        nc.vector.tensor_copy(parts[:], pd[0:97, :])
                        pd2 = ops.tile([P, 512], f32, tag=f"po{qc}",
                                       name="pd2")
                        nc.tensor.matmul(pd2[0:1, :], sel_sb[0:97, 0:1],
                                         parts[:], start=True, stop=True)
                        dsb = dpool.tile([1, 512], f32, tag="d", name="dsb")
                        nc.vector.tensor_copy(dsb[:], pd2[0:1, :])
                        nc.sync.dma_start(den_d[h:h + 1, sl], dsb[:])

                # Same-head PV at lag 3: PV(h) j runs while ACT drains
                # EXP(h) j+1..j+3, so the PE tracks ACT with only a ~3-j
                # PV tail per head.  pv_den(h-1) is emitted at j==3 of
                # head h (its exps finished a full block ago, so it never
                # stalls), IMMEDIATELY BEFORE po(h)'s allocation: the
                # shared po/pd psum-bank WAR chain then follows emission
                # order (osb(h-1) -> pd(h-1) -> pd2(h-1) -> po(h)), so no
                # deadlock.
                LAG = 3

                def pv_j(h, j, po, Ph):
                    for qc in range(4):
                        nc.tensor.matmul(
                            po[qc][:], v2_sb[:, h, j, :],
                            Ph[j][:, qc * 512:(qc + 1) * 512],
                            start=(j == 0), stop=(j == SB - 1))

                prev = None
                for h in range(HPG):
                    Ph = []
                    po = None
                    for j in range(SB):
                        Pj = ppool.tile([P, S], bf16, tag="P", name="Pj")
                        scores_j(h, j, Pj)
                        Ph.append(Pj)
                        if j == LAG:
                            if prev is not None:
                                pv_den(prev[0], prev[1], prev[2])
                            po = [ops.tile([P, 512], f32, tag=f"po{qc}",
                                           name=f"po{qc}") for qc in range(4)]
                        if j >= LAG:
                            pv_j(h, j - LAG, po, Ph)
                    for j in range(SB - LAG, SB):
                        pv_j(h, j, po, Ph)
                    prev = (h, Ph, po)
                pv_den(prev[0], prev[1], prev[2])

    _split_excess_waits(nc, mybir)
    return nc


def _split_excess_waits(nc, mybir):
    """Each TPB instruction has ONE wait slot (NEURON_ISA_TPB_EVENTS); walrus
    refuses instructions with more sync waits.  Tile attaches the full
    vector-clock wait list to instructions, so split all but one wait out
    into standalone EventSemaphore (CTRL) instructions on the same engine,
    placed immediately before.  Semantics are identical: all waits must be
    satisfied before the instruction executes."""
    import copy
    template = None
    for blk in nc.m.functions[0].blocks:
        for inst in blk.instructions:
            if isinstance(inst, mybir.InstEventSemaphore):
                template = inst
                break
        if template is not None:
            break
    assert template is not None, "no EventSemaphore template found"
    uid = [0]
    for fn in nc.m.functions:
        for blk in fn.blocks:
            out = []
            for inst in blk.instructions:
                si = inst.sync_info
                if si is not None and len(si.on_wait) > 1:
                    waits = list(si.on_wait)
                    for w in waits[:-1]:
                        ev = copy.deepcopy(template)
                        ev.name = f"swsplit-{uid[0]}"
                        uid[0] += 1
                        ev.engine = inst.engine
                        ev.sync_info = mybir.SyncInfo(on_wait=[w], on_update=[])
                        out.append(ev)
                    si.on_wait = waits[-1:]
                    inst.sync_info = si
                out.append(inst)
            blk.instructions[:] = out
    return nc


def _numpy_fallback(x, attn_mask, Wqkv, bqkv, Wq, bq, Wv, bv):
    x = np.asarray(x, np.float32)
    qkv = x @ np.asarray(Wqkv, np.float32) + np.asarray(bqkv, np.float32)
    q, k, v = np.split(qkv, 3, axis=-1)
    q = q.reshape(B, S, G, HPG, HD)
    k = k.reshape(B, S, G, HPG, HD)
    v = v.reshape(B, S, G, HPG, HD)
    q = np.einsum('bsghd,gde->bsghe', q, np.asarray(Wq, np.float32)) \
        + np.asarray(bq, np.float32)[None, None, :, None, :]
    v = np.einsum('bsghd,gde->bsghe', v, np.asarray(Wv, np.float32)) \
        + np.asarray(bv, np.float32)[None, None, :, None, :]
    out = np.empty((B, S, G, HPG, HD), np.float32)
    for b in range(B):
        for g in range(G):
            for hh in range(HPG):
                s = (q[b, :, g, hh] @ k[b, :, g, hh].T) * SCALE
                s = s - s.max(axis=-1, keepdims=True)
                p = np.exp(s)
                p /= p.sum(axis=-1, keepdims=True)
                p = p * np.asarray(attn_mask, np.float32)
                out[b, :, g, hh] = p @ v[b, :, g, hh]
    return out.reshape(B, S, D)


def kernel(x, attn_mask, Wqkv, bqkv, Wq, bq, Wv, bv):
    x = np.asarray(x)
    attn_mask = np.asarray(attn_mask)
    Wqkv = np.asarray(Wqkv)
    bqkv = np.asarray(bqkv)
    Wq = np.asarray(Wq)
    bq = np.asarray(bq)
    Wv = np.asarray(Wv)
    bv = np.asarray(bv)

    if not np.all(attn_mask == 1.0):
        # general (non-ones) post-softmax mask: correct but slow host path
        return _numpy_fallback(x, attn_mask, Wqkv, bqkv, Wq, bq, Wv, bv)

    if "nc" not in _CACHE:
        _CACHE["nc"] = _build_program()
    nc = _CACHE["nc"]
    from concourse.bass_utils import run_bass_kernel_spmd

    bf = ml_dtypes.bfloat16
    sel_np = np.zeros((P, 1), np.float32)
    sel_np[0::32] = 1.0
    in_maps = []
    # xt layout [chunk, p, ko, s']: xt[c,p,ko,s'] = x[b][c*512+s', ko*128+p]
    x_bf = []
    for b in range(B):
        xT = np.asarray(x[b], np.float32).T.astype(bf)      # [D, S]
        x_bf.append(np.ascontiguousarray(
            xT.reshape(KB, P, NCORES // 2, 512).transpose(2, 1, 0, 3)))
    Wq32 = np.asarray(Wq, np.float32)
    Wv32 = np.asarray(Wv, np.float32)
    host_bias = []

    def pmajor(w):
        """[D, N] -> [P, KB, N] with [p, ko, n] = w[ko*128+p, n]"""
        return np.ascontiguousarray(
            w.reshape(KB, P, w.shape[1]).transpose(1, 0, 2))

    for c in range(NCORES):
        b, g = divmod(c, G)
        cols = slice(g * GC, (g + 1) * GC)
        wq_c = Wqkv[:, 0 * D:1 * D][:, cols].astype(np.float32)
        wk_c = Wqkv[:, 1 * D:2 * D][:, cols]
        wv_c = Wqkv[:, 2 * D:3 * D][:, cols]
        # fold the per-group query projection + attention scale on host:
        wqs = Wq32[g] * SCALE
        wq_fold = (wq_c.reshape(D, HPG, HD) @ wqs[None]).reshape(D, GC)
        wqk = np.concatenate([wq_fold.astype(bf), np.asarray(wk_c, bf)],
                             axis=1)
        bq1 = bqkv[0 * D:1 * D][cols].astype(np.float32)
        bk1 = bqkv[1 * D:2 * D][cols].astype(np.float32)
        bv1 = bqkv[2 * D:3 * D][cols].astype(np.float32)
        bq2 = (bq1.reshape(HPG, HD) @ wqs
               + np.asarray(bq, np.float32)[g] * SCALE).reshape(GC)
        b1cat = np.concatenate([bq2, bk1]).astype(np.float32)
        host_bias.append(
            (bv1.reshape(HPG, HD) @ Wv32[g]
             + np.asarray(bv, np.float32)[g][None, :]).reshape(GC))
        in_maps.append({
            "xt": x_bf[b],
            "wqk": pmajor(wqk),
            "wv1": pmajor(np.asarray(wv_c, bf)),
            "b1": np.ascontiguousarray(b1cat.reshape(8, P).T),
            "wv": np.ascontiguousarray(Wv32[g].astype(bf)),
            "onesc": np.ones((P, 1), bf),
            "sel": sel_np,
        })

    res = run_bass_kernel_spmd(nc, in_maps, list(range(NCORES)),
                               **_CACHE.get("run_kwargs", {}))
    _CACHE["last_results"] = res

    out = np.empty((B, S, D), np.float32)
    for c in range(NCORES):
        b, g = divmod(c, G)
        o = res.results[c]["out"]          # [GC, S] unnormalized out^T
        den = res.results[c]["den"]        # [HPG, S]
        o = o / np.repeat(den, HD, axis=0)  # normalize rows h*128+e by den[h]
        o = o + host_bias[c][:, None]
        out[b, :, g * GC:(g + 1) * GC] = o.T
    return out


# revision 28
# speedup vs baseline: 1.3377x; 1.0457x over previous
"""GQA dense-transformer kernel for 8 Trainium2 NeuronCores.

Problem (hardcoded): B=2, S=2048, D=2048, kv_heads=16, groups G=4, HPG=4,
HD=128.  reference:
    qkv = x @ Wqkv + bqkv ; q,k,v = split(qkv)
    q = einsum('bsghd,gde->bsghe', q, Wq) + bq   (per-group shared proj)
    v = einsum('bsghd,gde->bsghe', v, Wv) + bv
    scores = einsum('bqghd,bkghd->bghqk', q, k) / sqrt(HD)
    attn = softmax(scores) * attn_mask           (mask == ones at grading)
    out = einsum('bghqk,bkghd->bqghd', attn, v)  -> [B,S,D]

Sharding: core c = b*4 + g handles (batch b, group g): it computes the
512 output columns [g*512,(g+1)*512) of out[b].

Per-core device program (bf16 matmuls, fp32 PSUM):
  The per-group query projection Wq (and the attention scale) is folded
  into Wqkv on the HOST: W_q2'[:,hs] = Wqkv_q[:,hs] @ (Wq[g]*SCALE), so
  the device computes q2^T and k^T directly in one projection pass.
  All inputs are RELAID OUT partition-major on the host so every DMA
  descriptor covers a 4-16KB contiguous run (the dominant startup cost
  is per-descriptor overhead, not bytes).
  phase 1: chunk 0 (s in [0,512)) runs K-OUTER in groups of 4 k-blocks
           with 8 live PSUM banks (m=0..7 = q2',k), so the PE starts
           after ~1.5MB of DMA and then tracks the DMA stream; v
           (m=8..11) + chunks 1..3 run m-outer.  v2 natural blocks via
           lhsT=v1^T slices @ wv.  q2/k evacuate on ACT (bias add),
           v1/v2 on DVE.
  phase 2: software-pipelined across heads: for each j, scores(h) j
           (PE->PSUM, exp on ACT) is interleaved with PV(h-1) j
           (j-MAJOR accumulation into 4 simultaneous po banks), so the
           PE always has dense work while ACT drains the exps.
           Denominators per head via concurrent M=1 ones-matmuls in 4
           PE col-groups + one fp32r selector matmul (psum banks shared
           with po via tag rotation).  Output is UNNORMALIZED out^T +
           denominators; softmax division + v-path bias happen on host.
"""
import sys
import numpy as np

sys.path.insert(0, "/opt/trn_rl_repo")
import ml_dtypes  # noqa: E402

B, S, D = 2, 2048, 2048
G, HPG, HD = 4, 4, 128
GC = HPG * HD            # 512 columns per group
SCALE = HD ** -0.5
P = 128
KB = D // P              # 16 contraction blocks
SB = S // P              # 16 sk blocks
NCORES = 8

_CACHE: dict = {}


def _build_program():
    import concourse.tile_sem_assignment as tsa
    # Walrus caps sync waits per instruction; _split_excess_waits breaks
    # any multi-wait compute instruction into standalone EventSemaphore
    # CTRLs on the same engine.  Keep the default 8 HWDGE semaphores so
    # DMA-completion waits stay fine-grained (a single shared semaphore
    # makes every consumer wait on the cumulative count of all earlier
    # DMAs, which serializes the startup loads).
    tsa.NUM_HWDGE_SEMS = 8

    import concourse.bass as bass
    import concourse.tile as tile
    from concourse import mybir
    from contextlib import ExitStack

    bf16 = mybir.dt.bfloat16
    f32 = mybir.dt.float32

    nc = bass.Bass(trn_type="TRN2")
    SCH = 512                 # s-chunk width for projection phase
    NCH = S // SCH            # 4 chunks
    QCH = 1024                # sq chunk width for scores/exp
    # partition-major input layouts (see kernel() for the host side)
    xt_d = nc.dram_tensor("xt", [NCH, P, KB, SCH], bf16, kind="ExternalInput")
    wqk_d = nc.dram_tensor("wqk", [P, KB, 2 * GC], bf16, kind="ExternalInput")
    wv1_d = nc.dram_tensor("wv1", [P, KB, GC], bf16, kind="ExternalInput")
    b1_d = nc.dram_tensor("b1", [P, 8], f32, kind="ExternalInput")
    wv_d = nc.dram_tensor("wv", [HD, HD], bf16, kind="ExternalInput")
    onesc_d = nc.dram_tensor("onesc", [P, 1], bf16, kind="ExternalInput")
    sel_d = nc.dram_tensor("sel", [P, 1], f32, kind="ExternalInput")
    out_d = nc.dram_tensor("out", [GC, S], f32, kind="ExternalOutput")
    den_d = nc.dram_tensor("den", [HPG, S], f32, kind="ExternalOutput")

    Exp = mybir.ActivationFunctionType.Exp
    Ident = mybir.ActivationFunctionType.Identity

    with tile.TileContext(nc) as tc:
        with ExitStack() as octx:
            # ---- persistent tiles ----
            persist = octx.enter_context(tc.tile_pool(name="persist", bufs=1))
            k_sb = persist.tile([P, HPG, S], bf16)       # k^T per head
            q2_sb = persist.tile([P, HPG, S], bf16)      # q2^T per head
            v2_sb = persist.tile([P, HPG, SB, HD], bf16)  # v2 natural blocks
            wv_sb = persist.tile([HD, HD], bf16)
            b1_sb = persist.tile([P, 8], f32)
            ones_sb = persist.tile([P, 1], bf16)
            sel_f = persist.tile([P, 1], f32)
            sel_sb = persist.tile([P, 1], mybir.dt.float32r)
            nc.sync.dma_start(wv_sb[:], wv_d[:])
            nc.sync.dma_start(b1_sb[:], b1_d[:])
            nc.sync.dma_start(ones_sb[:], onesc_d[:])
            nc.sync.dma_start(sel_f[:], sel_d[:])
            nc.vector.tensor_copy(sel_sb[:], sel_f[:])

            # ---------------- phase 1: projections ----------------
            with ExitStack() as ctx:
                wpool = ctx.enter_context(tc.tile_pool(name="w1", bufs=1))
                xpool = ctx.enter_context(tc.tile_pool(name="xT", bufs=2))
                tpool = ctx.enter_context(tc.tile_pool(name="tmp", bufs=4))
                wqk_sb = wpool.tile([P, KB, 2 * GC], bf16)
                wv1_sb = wpool.tile([P, KB, GC], bf16)
                xT0 = wpool.tile([P, KB, SCH], bf16)

                def wslice(m, k):
                    if m < 8:
                        return wqk_sb[:, k, m * P:(m + 1) * P]
                    return wv1_sb[:, k, (m - 8) * P:(m - 7) * P]

                # stage A: chunk 0, m=0..7 (q2', k), K-OUTER in groups of
                # 4 k-blocks with 8 live PSUM banks.  Each group's DMA is
                # one 128-descriptor transfer with 4-8KB contiguous runs.
                with ExitStack() as actx:
                    p0 = actx.enter_context(
                        tc.tile_pool(name="p0", bufs=1, space="PSUM"))
                    psA = [p0.tile([P, SCH], f32, tag=f"m{m}", name=f"psA{m}")
                           for m in range(8)]
                    for g4 in range(4):
                        ks = slice(4 * g4, 4 * g4 + 4)
                        nc.sync.dma_start(wqk_sb[:, ks], wqk_d[:, ks])
                        nc.sync.dma_start(xT0[:, ks], xt_d[0, :, ks])
                    nc.sync.dma_start(wv1_sb[:], wv1_d[:])
                    for k in range(KB):
                        for m in range(8):
                            nc.tensor.matmul(
                                psA[m][:], wslice(m, k), xT0[:, k],
                                start=(k == 0), stop=(k == KB - 1))
                    for m in range(8):
                        dst = q2_sb if m < 4 else k_sb
                        nc.scalar.activation(
                            dst[:, m % 4, 0:SCH], psA[m][:],
                            Ident, bias=b1_sb[:, m:m + 1])

                # stage B: v for chunk 0, then all m for chunks 1..3
                with ExitStack() as bctx:
                    pp = bctx.enter_context(
                        tc.tile_pool(name="pp", bufs=3, space="PSUM"))
                    pv = bctx.enter_context(
                        tc.tile_pool(name="pv", bufs=2, space="PSUM"))

                    def vblock(ps, h, c):
                        """v1 psum chunk -> v1 tmp -> v2 natural blocks."""
                        v1 = tpool.tile([P, SCH], bf16, tag="v1")
                        nc.vector.tensor_copy(v1[:], ps[:])
                        for sb in range(SCH // P):
                            ps3 = pv.tile([P, HD], f32)
                            nc.tensor.matmul(
                                ps3[:], v1[:, sb * P:(sb + 1) * P],
                                wv_sb[:], start=True, stop=True)
                            nc.vector.tensor_copy(
                                v2_sb[:, h, c * (SCH // P) + sb, :], ps3[:])

                    for m in range(8, 12):
                        ps = pp.tile([P, SCH], f32)
                        for k in range(KB):
                            nc.tensor.matmul(
                                ps[:], wslice(m, k), xT0[:, k],
                                start=(k == 0), stop=(k == KB - 1))
                        vblock(ps, m - 8, 0)

                    for c in range(1, NCH):
                        xT = xpool.tile([P, KB, SCH], bf16, tag="xT")
                        nc.sync.dma_start(xT[:], xt_d[c])
                        for m in range(12):
                            ps = pp.tile([P, SCH], f32)
                            for k in range(KB):
                                nc.tensor.matmul(
                                    ps[:], wslice(m, k), xT[:, k],
                                    start=(k == 0), stop=(k == KB - 1))
                            if m < 8:
                                dst = q2_sb if m < 4 else k_sb
                                nc.scalar.activation(
                                    dst[:, m % 4, c * SCH:(c + 1) * SCH],
                                    ps[:], Ident, bias=b1_sb[:, m:m + 1])
                            else:
                                vblock(ps, m - 8, c)

            # ---------------- phase 2: attention ----------------
            # software pipeline: per j, scores(h) j interleaves with the
            # j-MAJOR PV of the SAME head at lag 1 (PV j-1 runs while
            # EXP j drains), so the PE tracks ACT with no dense-PV tail.
            with ExitStack() as ctx:
                ppool = ctx.enter_context(tc.tile_pool(name="P", bufs=24))
                opool = ctx.enter_context(tc.tile_pool(name="osb", bufs=4))
                dpool = ctx.enter_context(tc.tile_pool(name="dsb", bufs=4))
                sps = ctx.enter_context(
                    tc.tile_pool(name="sps", bufs=2, space="PSUM"))
                ops = ctx.enter_context(
                    tc.tile_pool(name="ops", bufs=1, space="PSUM"))

                def scores_j(h, j, Pj):
                    for qc in range(S // QCH):
                        ss = sps.tile([P, QCH], f32, name="ss")
                        for half in range(QCH // 512):
                            off = qc * QCH + half * 512
                            nc.tensor.matmul(
                                ss[:, half * 512:(half + 1) * 512],
                                k_sb[:, h, j * P:(j + 1) * P],
                                q2_sb[:, h, off:off + 512],
                                start=True, stop=True)
                        nc.scalar.activation(
                            Pj[:, qc * QCH:(qc + 1) * QCH], ss[:], Exp)

                def pv_den(h, Ph, po):
                    """emit PV epilogue + denominators for head h."""
                    for qc in range(4):
                        sl = slice(qc * 512, (qc + 1) * 512)
                        osb = opool.tile([P, 512], f32, tag="o", name="osb")
                        nc.vector.tensor_copy(osb[:], po[qc][:])
                        nc.sync.dma_start(out_d[h * P:(h + 1) * P, sl], osb[:])
                        # denominators: 4 concurrent M=1 ones-matmuls in 4
                        # col-groups x 4 rounds, then fp32r selector matmul.
                        pd = ops.tile([P, 512], f32, tag=f"po{qc}", name="pd")
                        for r in range(4):
                            for jj in range(4):
                                j = r * 4 + jj
                                nc.tensor.matmul(
                                    pd[32 * jj:32 * jj + 1, :],
                                    ones_sb[:, 0:1], Ph[j][:, sl],
                                    start=(r == 0), stop=(r == 3),
                                    tile_position=(0, 32 * jj))
                        parts = dpool.tile([97, 512], mybir.dt.float32r,
                                           tag="dp", name="parts")
                        nc.vector.tensor_copy(parts[:], pd[0:97, :])
                        pd2 = ops.tile([P, 512], f32, tag=f"po{qc}",
                                       name="pd2")
                        nc.tensor.matmul(pd2[0:1, :], sel_sb[0:97, 0:1],
                                         parts[:], start=True, stop=True)
                        dsb = dpool.tile([1, 512], f32, tag="d", name="dsb")
                        nc.vector.tensor_copy(dsb[:], pd2[0:1, :])
                        nc.sync.dma_start(den_d[h:h + 1, sl], dsb[:])

                # Same-head PV at lag 3: PV(h) j runs while ACT drains
                # EXP(h) j+1..j+3, so the PE tracks ACT with only a ~3-j
                # PV tail per head.  pv_den(h-1) is emitted at j==3 of
                # head h (its exps finished a full block ago, so it never
                # stalls), IMMEDIATELY BEFORE po(h)'s allocation: the
                # shared po/pd psum-bank WAR chain then follows emission
                # order (osb(h-1) -> pd(h-1) -> pd2(h-1) -> po(h)), so no
                # deadlock.
                LAG = 3

                def pv_j(h, j, po, Ph):
                    for qc in range(4):
                        nc.tensor.matmul(
                            po[qc][:], v2_sb[:, h, j, :],
                            Ph[j][:, qc * 512:(qc + 1) * 512],
                            start=(j == 0), stop=(j == SB - 1))

                prev = None
                for h in range(HPG):
                    Ph = []
                    po = None
                    for j in range(SB):
                        Pj = ppool.tile([P, S], bf16, tag="P", name="Pj")
                        scores_j(h, j, Pj)
                        Ph.append(Pj)
                        if j == LAG:
                            if prev is not None:
                                pv_den(prev[0], prev[1], prev[2])
                            po = [ops.tile([P, 512], f32, tag=f"po{qc}",
                                           name=f"po{qc}") for qc in range(4)]
                        if j >= LAG:
                            pv_j(h, j - LAG, po, Ph)
                    for j in range(SB - LAG, SB):
                        pv_j(h, j, po, Ph)
                    prev = (h, Ph, po)
                pv_den(prev[0], prev[1], prev[2])

    _split_excess_waits(nc, mybir)
    return nc


def _split_excess_waits(nc, mybir):
    """Each TPB instruction has ONE wait slot (NEURON_ISA_TPB_EVENTS); walrus
    refuses instructions with more sync waits.  Tile attaches the full
    vector-clock wait list to instructions, so split all but one wait out
    into standalone EventSemaphore (CTRL) instructions on the same engine,
    placed immediately before.  Semantics are identical: all waits must be
    satisfied before the instruction executes."""
    import copy
    template = None
    for blk in nc.m.functions[0].blocks:
        for inst in blk.instructions:
            if isinstance(inst, mybir.InstEventSemaphore):
                template = inst
                break
        if template is not None:
            break
    assert template is not None, "no EventSemaphore template found"
    uid = [0]
    for fn in nc.m.functions:
        for blk in fn.blocks:
            out = []
            for inst in blk.instructions:
                si = inst.sync_info
                if si is not None and len(si.on_wait) > 1:
                    waits = list(si.on_wait)
                    for w in waits[:-1]:
                        ev = copy.deepcopy(template)
                        ev.name = f"swsplit-{uid[0]}"
                        uid[0] += 1
                        ev.engine = inst.engine
                        ev.sync_info = mybir.SyncInfo(on_wait=[w], on_update=[])
                        out.append(ev)
                    si.on_wait = waits[-1:]
                    inst.sync_info = si
                out.append(inst)
            blk.instructions[:] = out
    return nc


def _numpy_fallback(x, attn_mask, Wqkv, bqkv, Wq, bq, Wv, bv):
    x = np.asarray(x, np.float32)
    qkv = x @ np.asarray(Wqkv, np.float32) + np.asarray(bqkv, np.float32)
    q, k, v = np.split(qkv, 3, axis=-1)
    q = q.reshape(B, S, G, HPG, HD)
    k = k.reshape(B, S, G, HPG, HD)
    v = v.reshape(B, S, G, HPG, HD)
    q = np.einsum('bsghd,gde->bsghe', q, np.asarray(Wq, np.float32)) \
        + np.asarray(bq, np.float32)[None, None, :, None, :]
    v = np.einsum('bsghd,gde->bsghe', v, np.asarray(Wv, np.float32)) \
        + np.asarray(bv, np.float32)[None, None, :, None, :]
    out = np.empty((B, S, G, HPG, HD), np.float32)
    for b in range(B):
        for g in range(G):
            for hh in range(HPG):
                s = (q[b, :, g, hh] @ k[b, :, g, hh].T) * SCALE
                s = s - s.max(axis=-1, keepdims=True)
                p = np.exp(s)
                p /= p.sum(axis=-1, keepdims=True)
                p = p * np.asarray(attn_mask, np.float32)
                out[b, :, g, hh] = p @ v[b, :, g, hh]
    return out.reshape(B, S, D)


def kernel(x, attn_mask, Wqkv, bqkv, Wq, bq, Wv, bv):
    x = np.asarray(x)
    attn_mask = np.asarray(attn_mask)
    Wqkv = np.asarray(Wqkv)
    bqkv = np.asarray(bqkv)
    Wq = np.asarray(Wq)
    bq = np.asarray(bq)
    Wv = np.asarray(Wv)
    bv = np.asarray(bv)

    if not np.all(attn_mask == 1.0):
        # general (non-ones) post-softmax mask: correct but slow host path
        return _numpy_fallback(x, attn_mask, Wqkv, bqkv, Wq, bq, Wv, bv)

    if "nc" not in _CACHE:
        _CACHE["nc"] = _build_program()
    nc = _CACHE["nc"]
    from concourse.bass_utils import run_bass_kernel_spmd

    bf = ml_dtypes.bfloat16
    sel_np = np.zeros((P, 1), np.float32)
    sel_np[0::32] = 1.0
    in_maps = []
    # xt layout [chunk, p, ko, s']: xt[c,p,ko,s'] = x[b][c*512+s', ko*128+p]
    x_bf = []
    for b in range(B):
        xT = np.asarray(x[b], np.float32).T.astype(bf)      # [D, S]
        x_bf.append(np.ascontiguousarray(
            xT.reshape(KB, P, NCORES // 2, 512).transpose(2, 1, 0, 3)))
    Wq32 = np.asarray(Wq, np.float32)
    Wv32 = np.asarray(Wv, np.float32)
    host_bias = []

    def pmajor(w):
        """[D, N] -> [P, KB, N] with [p, ko, n] = w[ko*128+p, n]"""
        return np.ascontiguousarray(
            w.reshape(KB, P, w.shape[1]).transpose(1, 0, 2))

    for c in range(NCORES):
        b, g = divmod(c, G)
        cols = slice(g * GC, (g + 1) * GC)
        wq_c = Wqkv[:, 0 * D:1 * D][:, cols].astype(np.float32)
        wk_c = Wqkv[:, 1 * D:2 * D][:, cols]
        wv_c = Wqkv[:, 2 * D:3 * D][:, cols]
        # fold the per-group query projection + attention scale on host:
        wqs = Wq32[g] * SCALE
        wq_fold = (wq_c.reshape(D, HPG, HD) @ wqs[None]).reshape(D, GC)
        wqk = np.concatenate([wq_fold.astype(bf), np.asarray(wk_c, bf)],
                             axis=1)
        bq1 = bqkv[0 * D:1 * D][cols].astype(np.float32)
        bk1 = bqkv[1 * D:2 * D][cols].astype(np.float32)
        bv1 = bqkv[2 * D:3 * D][cols].astype(np.float32)
        bq2 = (bq1.reshape(HPG, HD) @ wqs
               + np.asarray(bq, np.float32)[g] * SCALE).reshape(GC)
        b1cat = np.concatenate([bq2, bk1]).astype(np.float32)
        host_bias.append(
            (bv1.reshape(HPG, HD) @ Wv32[g]
             + np.asarray(bv, np.float32)[g][None, :]).reshape(GC))
        in_maps.append({
            "xt": x_bf[b],
            "wqk": pmajor(wqk),
            "wv1": pmajor(np.asarray(wv_c, bf)),
            "b1": np.ascontiguousarray(b1cat.reshape(8, P).T),
            "wv": np.ascontiguousarray(Wv32[g].astype(bf)),
            "onesc": np.ones((P, 1), bf),
            "sel": sel_np,
        })

    res = run_bass_kernel_spmd(nc, in_maps, list(range(NCORES)),
                               **_CACHE.get("run_kwargs", {}))
    _CACHE["last_results"] = res

    out = np.empty((B, S, D), np.float32)
    for c in range(NCORES):
        b, g = divmod(c, G)
        o = res.results[c]["out"]          # [GC, S] unnormalized out^T
        den = res.results[c]["den"]        # [HPG, S]
        o = o / np.repeat(den, HD, axis=0)  # normalize rows h*128+e by den[h]
        o = o + host_bias[c][:, None]
        out[b, :, g * GC:(g + 1) * GC] = o.T
    return out


# revision 32
# speedup vs baseline: 1.3472x; 1.0071x over previous
"""GQA dense-transformer kernel for 8 Trainium2 NeuronCores.

Problem (hardcoded): B=2, S=2048, D=2048, kv_heads=16, groups G=4, HPG=4,
HD=128.  reference:
    qkv = x @ Wqkv + bqkv ; q,k,v = split(qkv)
    q = einsum('bsghd,gde->bsghe', q, Wq) + bq   (per-group shared proj)
    v = einsum('bsghd,gde->bsghe', v, Wv) + bv
    scores = einsum('bqghd,bkghd->bghqk', q, k) / sqrt(HD)
    attn = softmax(scores) * attn_mask           (mask == ones at grading)
    out = einsum('bghqk,bkghd->bqghd', attn, v)  -> [B,S,D]

Sharding: core c = b*4 + g handles (batch b, group g): it computes the
512 output columns [g*512,(g+1)*512) of out[b].

Per-core device program (bf16 matmuls, fp32 PSUM):
  The per-group query projection Wq (and the attention scale) is folded
  into Wqkv on the HOST: W_q2'[:,hs] = Wqkv_q[:,hs] @ (Wq[g]*SCALE), so
  the device computes q2^T and k^T directly in one projection pass.
  All inputs are RELAID OUT partition-major on the host so every DMA
  descriptor covers a 4-16KB contiguous run (the dominant startup cost
  is per-descriptor overhead, not bytes).
  phase 1: chunk 0 (s in [0,512)) runs K-OUTER in groups of 4 k-blocks
           with 8 live PSUM banks (m=0..7 = q2',k), so the PE starts
           after ~1.5MB of DMA and then tracks the DMA stream; v
           (m=8..11) + chunks 1..3 run m-outer.  v2 natural blocks via
           lhsT=v1^T slices @ wv.  q2/k evacuate on ACT (bias add),
           v1/v2 on DVE.
  phase 2: software-pipelined across heads: for each j, scores(h) j
           (PE->PSUM, exp on ACT) is interleaved with PV(h-1) j
           (j-MAJOR accumulation into 4 simultaneous po banks), so the
           PE always has dense work while ACT drains the exps.
           Denominators per head via concurrent M=1 ones-matmuls in 4
           PE col-groups + one fp32r selector matmul (psum banks shared
           with po via tag rotation).  Output is UNNORMALIZED out^T +
           denominators; softmax division + v-path bias happen on host.
"""
import sys
import numpy as np

sys.path.insert(0, "/opt/trn_rl_repo")
import ml_dtypes  # noqa: E402

B, S, D = 2, 2048, 2048
G, HPG, HD = 4, 4, 128
GC = HPG * HD            # 512 columns per group
SCALE = HD ** -0.5
P = 128
KB = D // P              # 16 contraction blocks
SB = S // P              # 16 sk blocks
NCORES = 8

_CACHE: dict = {}


def _build_program():
    import concourse.tile_sem_assignment as tsa
    # Walrus caps sync waits per instruction; _split_excess_waits breaks
    # any multi-wait compute instruction into standalone EventSemaphore
    # CTRLs on the same engine.  Keep the default 8 HWDGE semaphores so
    # DMA-completion waits stay fine-grained (a single shared semaphore
    # makes every consumer wait on the cumulative count of all earlier
    # DMAs, which serializes the startup loads).
    tsa.NUM_HWDGE_SEMS = 8

    import concourse.bass as bass
    import concourse.tile as tile
    from concourse import mybir
    from contextlib import ExitStack

    bf16 = mybir.dt.bfloat16
    f32 = mybir.dt.float32

    nc = bass.Bass(trn_type="TRN2")
    SCH = 512                 # s-chunk width for projection phase
    NCH = S // SCH            # 4 chunks
    QCH = 1024                # sq chunk width for scores/exp
    # partition-major input layouts (see kernel() for the host side)
    xt_d = nc.dram_tensor("xt", [NCH, P, KB, SCH], bf16, kind="ExternalInput")
    wqk_d = nc.dram_tensor("wqk", [P, KB, 2 * GC], bf16, kind="ExternalInput")
    wv1_d = nc.dram_tensor("wv1", [P, KB, GC], bf16, kind="ExternalInput")
    b1_d = nc.dram_tensor("b1", [P, 8], f32, kind="ExternalInput")
    wv_d = nc.dram_tensor("wv", [HD, HD], bf16, kind="ExternalInput")
    onesc_d = nc.dram_tensor("onesc", [P, 1], bf16, kind="ExternalInput")
    sel_d = nc.dram_tensor("sel", [P, 1], f32, kind="ExternalInput")
    out_d = nc.dram_tensor("out", [GC, S], bf16, kind="ExternalOutput")
    den_d = nc.dram_tensor("den", [HPG, S], f32, kind="ExternalOutput")

    Exp = mybir.ActivationFunctionType.Exp
    Ident = mybir.ActivationFunctionType.Identity

    with tile.TileContext(nc) as tc:
        with ExitStack() as octx:
            # ---- persistent tiles ----
            persist = octx.enter_context(tc.tile_pool(name="persist", bufs=1))
            k_sb = persist.tile([P, HPG, S], bf16)       # k^T per head
            q2_sb = persist.tile([P, HPG, S], bf16)      # q2^T per head
            v2_sb = persist.tile([P, HPG, SB, HD], bf16)  # v2 natural blocks
            wv_sb = persist.tile([HD, HD], bf16)
            b1_sb = persist.tile([P, 8], f32)
            ones_sb = persist.tile([P, 1], bf16)
            sel_f = persist.tile([P, 1], f32)
            sel_sb = persist.tile([P, 1], mybir.dt.float32r)
            nc.sync.dma_start(wv_sb[:], wv_d[:])
            nc.sync.dma_start(b1_sb[:], b1_d[:])
            nc.sync.dma_start(ones_sb[:], onesc_d[:])
            nc.sync.dma_start(sel_f[:], sel_d[:])
            nc.vector.tensor_copy(sel_sb[:], sel_f[:])

            # ---------------- phase 1: projections ----------------
            with ExitStack() as ctx:
                wpool = ctx.enter_context(tc.tile_pool(name="w1", bufs=1))
                xpool = ctx.enter_context(tc.tile_pool(name="xT", bufs=2))
                tpool = ctx.enter_context(tc.tile_pool(name="tmp", bufs=4))
                wqk_sb = wpool.tile([P, KB, 2 * GC], bf16)
                wv1_sb = wpool.tile([P, KB, GC], bf16)
                xT0 = wpool.tile([P, KB, SCH], bf16)

                def wslice(m, k):
                    if m < 8:
                        return wqk_sb[:, k, m * P:(m + 1) * P]
                    return wv1_sb[:, k, (m - 8) * P:(m - 7) * P]

                # stage A: chunk 0, m=0..7 (q2', k), K-OUTER in groups of
                # 4 k-blocks with 8 live PSUM banks.  Each group's DMA is
                # one 128-descriptor transfer with 4-8KB contiguous runs.
                with ExitStack() as actx:
                    p0 = actx.enter_context(
                        tc.tile_pool(name="p0", bufs=1, space="PSUM"))
                    psA = [p0.tile([P, SCH], f32, tag=f"m{m}", name=f"psA{m}")
                           for m in range(8)]
                    # first k-groups are small so the first matmul waits on
                    # ~0.4MB, not 1.5MB (single-DMA throughput ~40-60GB/s)
                    kgroups = [(0, 1), (1, 2), (2, 4), (4, 8), (8, 12),
                               (12, 16)]
                    for lo, hi in kgroups:
                        ks = slice(lo, hi)
                        nc.sync.dma_start(wqk_sb[:, ks], wqk_d[:, ks])
                        nc.sync.dma_start(xT0[:, ks], xt_d[0, :, ks])
                    nc.sync.dma_start(wv1_sb[:], wv1_d[:])
                    for k in range(KB):
                        for m in range(8):
                            nc.tensor.matmul(
                                psA[m][:], wslice(m, k), xT0[:, k],
                                start=(k == 0), stop=(k == KB - 1))
                    for m in range(8):
                        dst = q2_sb if m < 4 else k_sb
                        nc.scalar.activation(
                            dst[:, m % 4, 0:SCH], psA[m][:],
                            Ident, bias=b1_sb[:, m:m + 1])

                # stage B: v for chunk 0, then all m for chunks 1..3
                with ExitStack() as bctx:
                    pp = bctx.enter_context(
                        tc.tile_pool(name="pp", bufs=3, space="PSUM"))
                    pv = bctx.enter_context(
                        tc.tile_pool(name="pv", bufs=2, space="PSUM"))

                    def vblock(ps, h, c):
                        """v1 psum chunk -> v1 tmp -> v2 natural blocks."""
                        v1 = tpool.tile([P, SCH], bf16, tag="v1")
                        nc.vector.tensor_copy(v1[:], ps[:])
                        for sb in range(SCH // P):
                            ps3 = pv.tile([P, HD], f32)
                            nc.tensor.matmul(
                                ps3[:], v1[:, sb * P:(sb + 1) * P],
                                wv_sb[:], start=True, stop=True)
                            nc.vector.tensor_copy(
                                v2_sb[:, h, c * (SCH // P) + sb, :], ps3[:])

                    for m in range(8, 12):
                        ps = pp.tile([P, SCH], f32)
                        for k in range(KB):
                            nc.tensor.matmul(
                                ps[:], wslice(m, k), xT0[:, k],
                                start=(k == 0), stop=(k == KB - 1))
                        vblock(ps, m - 8, 0)

                    for c in range(1, NCH):
                        xT = xpool.tile([P, KB, SCH], bf16, tag="xT")
                        nc.sync.dma_start(xT[:], xt_d[c])
                        for m in range(12):
                            ps = pp.tile([P, SCH], f32)
                            for k in range(KB):
                                nc.tensor.matmul(
                                    ps[:], wslice(m, k), xT[:, k],
                                    start=(k == 0), stop=(k == KB - 1))
                            if m < 8:
                                dst = q2_sb if m < 4 else k_sb
                                nc.scalar.activation(
                                    dst[:, m % 4, c * SCH:(c + 1) * SCH],
                                    ps[:], Ident, bias=b1_sb[:, m:m + 1])
                            else:
                                vblock(ps, m - 8, c)

            # ---------------- phase 2: attention ----------------
            # software pipeline: per j, scores(h) j interleaves with the
            # j-MAJOR PV of the SAME head at lag 1 (PV j-1 runs while
            # EXP j drains), so the PE tracks ACT with no dense-PV tail.
            with ExitStack() as ctx:
                ppool = ctx.enter_context(tc.tile_pool(name="P", bufs=24))
                opool = ctx.enter_context(tc.tile_pool(name="osb", bufs=4))
                dpool = ctx.enter_context(tc.tile_pool(name="dsb", bufs=4))
                sps = ctx.enter_context(
                    tc.tile_pool(name="sps", bufs=2, space="PSUM"))
                ops = ctx.enter_context(
                    tc.tile_pool(name="ops", bufs=1, space="PSUM"))

                def scores_j(h, j, Pj):
                    for qc in range(S // QCH):
                        ss = sps.tile([P, QCH], f32, name="ss")
                        for half in range(QCH // 512):
                            off = qc * QCH + half * 512
                            nc.tensor.matmul(
                                ss[:, half * 512:(half + 1) * 512],
                                k_sb[:, h, j * P:(j + 1) * P],
                                q2_sb[:, h, off:off + 512],
                                start=True, stop=True)
                        nc.scalar.activation(
                            Pj[:, qc * QCH:(qc + 1) * QCH], ss[:], Exp)

                def pv_den(h, Ph, po):
                    """emit PV epilogue + denominators for head h."""
                    for qc in range(4):
                        sl = slice(qc * 512, (qc + 1) * 512)
                        osb = opool.tile([P, 512], bf16, tag="o", name="osb")
                        nc.vector.tensor_copy(osb[:], po[qc][:])
                        nc.sync.dma_start(out_d[h * P:(h + 1) * P, sl], osb[:])
                        # denominators: 4 concurrent M=1 ones-matmuls in 4
                        # col-groups x 4 rounds, then fp32r selector matmul.
                        pd = ops.tile([P, 512], f32, tag=f"po{qc}", name="pd")
                        for r in range(4):
                            for jj in range(4):
                                j = r * 4 + jj
                                nc.tensor.matmul(
                                    pd[32 * jj:32 * jj + 1, :],
                                    ones_sb[:, 0:1], Ph[j][:, sl],
                                    start=(r == 0), stop=(r == 3),
                                    tile_position=(0, 32 * jj))
                        parts = dpool.tile([97, 512], mybir.dt.float32r,
                                           tag="dp", name="parts")
                        nc.vector.tensor_copy(parts[:], pd[0:97, :])
                        pd2 = ops.tile([P, 512], f32, tag=f"po{qc}",
                                       name="pd2")
                        nc.tensor.matmul(pd2[0:1, :], sel_sb[0:97, 0:1],
                                         parts[:], start=True, stop=True)
                        dsb = dpool.tile([1, 512], f32, tag="d", name="dsb")
                        nc.vector.tensor_copy(dsb[:], pd2[0:1, :])
                        nc.sync.dma_start(den_d[h:h + 1, sl], dsb[:])

                # Same-head PV at lag 3: PV(h) j runs while ACT drains
                # EXP(h) j+1..j+3, so the PE tracks ACT with only a ~3-j
                # PV tail per head.  pv_den(h-1) is emitted at j==3 of
                # head h (its exps finished a full block ago, so it never
                # stalls), IMMEDIATELY BEFORE po(h)'s allocation: the
                # shared po/pd psum-bank WAR chain then follows emission
                # order (osb(h-1) -> pd(h-1) -> pd2(h-1) -> po(h)), so no
                # deadlock.
                LAG = 3

                def pv_j(h, j, po, Ph):
                    for qc in range(4):
                        nc.tensor.matmul(
                            po[qc][:], v2_sb[:, h, j, :],
                            Ph[j][:, qc * 512:(qc + 1) * 512],
                            start=(j == 0), stop=(j == SB - 1))

                prev = None
                for h in range(HPG):
                    Ph = []
                    po = None
                    for j in range(SB):
                        Pj = ppool.tile([P, S], bf16, tag="P", name="Pj")
                        scores_j(h, j, Pj)
                        Ph.append(Pj)
                        if j == LAG:
                            if prev is not None:
                                pv_den(prev[0], prev[1], prev[2])
                            po = [ops.tile([P, 512], f32, tag=f"po{qc}",
                                           name=f"po{qc}") for qc in range(4)]
                        if j >= LAG:
                            pv_j(h, j - LAG, po, Ph)
                    for j in range(SB - LAG, SB):
                        pv_j(h, j, po, Ph)
                    prev = (h, Ph, po)
                pv_den(prev[0], prev[1], prev[2])

    _split_excess_waits(nc, mybir)
    return nc


def _split_excess_waits(nc, mybir):
    """Each TPB instruction has ONE wait slot (NEURON_ISA_TPB_EVENTS); walrus
    refuses instructions with more sync waits.  Tile attaches the full
    vector-clock wait list to instructions, so split all but one wait out
    into standalone EventSemaphore (CTRL) instructions on the same engine,
    placed immediately before.  Semantics are identical: all waits must be
    satisfied before the instruction executes."""
    import copy
    template = None
    for blk in nc.m.functions[0].blocks:
        for inst in blk.instructions:
            if isinstance(inst, mybir.InstEventSemaphore):
                template = inst
                break
        if template is not None:
            break
    assert template is not None, "no EventSemaphore template found"
    uid = [0]
    for fn in nc.m.functions:
        for blk in fn.blocks:
            out = []
            for inst in blk.instructions:
                si = inst.sync_info
                if si is not None and len(si.on_wait) > 1:
                    waits = list(si.on_wait)
                    for w in waits[:-1]:
                        ev = copy.deepcopy(template)
                        ev.name = f"swsplit-{uid[0]}"
                        uid[0] += 1
                        ev.engine = inst.engine
                        ev.sync_info = mybir.SyncInfo(on_wait=[w], on_update=[])
                        out.append(ev)
                    si.on_wait = waits[-1:]
                    inst.sync_info = si
                out.append(inst)
            blk.instructions[:] = out
    return nc


def _numpy_fallback(x, attn_mask, Wqkv, bqkv, Wq, bq, Wv, bv):
    x = np.asarray(x, np.float32)
    qkv = x @ np.asarray(Wqkv, np.float32) + np.asarray(bqkv, np.float32)
    q, k, v = np.split(qkv, 3, axis=-1)
    q = q.reshape(B, S, G, HPG, HD)
    k = k.reshape(B, S, G, HPG, HD)
    v = v.reshape(B, S, G, HPG, HD)
    q = np.einsum('bsghd,gde->bsghe', q, np.asarray(Wq, np.float32)) \
        + np.asarray(bq, np.float32)[None, None, :, None, :]
    v = np.einsum('bsghd,gde->bsghe', v, np.asarray(Wv, np.float32)) \
        + np.asarray(bv, np.float32)[None, None, :, None, :]
    out = np.empty((B, S, G, HPG, HD), np.float32)
    for b in range(B):
        for g in range(G):
            for hh in range(HPG):
                s = (q[b, :, g, hh] @ k[b, :, g, hh].T) * SCALE
                s = s - s.max(axis=-1, keepdims=True)
                p = np.exp(s)
                p /= p.sum(axis=-1, keepdims=True)
                p = p * np.asarray(attn_mask, np.float32)
                out[b, :, g, hh] = p @ v[b, :, g, hh]
    return out.reshape(B, S, D)


def kernel(x, attn_mask, Wqkv, bqkv, Wq, bq, Wv, bv):
    x = np.asarray(x)
    attn_mask = np.asarray(attn_mask)
    Wqkv = np.asarray(Wqkv)
    bqkv = np.asarray(bqkv)
    Wq = np.asarray(Wq)
    bq = np.asarray(bq)
    Wv = np.asarray(Wv)
    bv = np.asarray(bv)

    if not np.all(attn_mask == 1.0):
        # general (non-ones) post-softmax mask: correct but slow host path
        return _numpy_fallback(x, attn_mask, Wqkv, bqkv, Wq, bq, Wv, bv)

    if "nc" not in _CACHE:
        _CACHE["nc"] = _build_program()
    nc = _CACHE["nc"]
    from concourse.bass_utils import run_bass_kernel_spmd

    bf = ml_dtypes.bfloat16
    sel_np = np.zeros((P, 1), np.float32)
    sel_np[0::32] = 1.0
    in_maps = []
    # xt layout [chunk, p, ko, s']: xt[c,p,ko,s'] = x[b][c*512+s', ko*128+p]
    x_bf = []
    for b in range(B):
        xT = np.asarray(x[b], np.float32).T.astype(bf)      # [D, S]
        x_bf.append(np.ascontiguousarray(
            xT.reshape(KB, P, NCORES // 2, 512).transpose(2, 1, 0, 3)))
    Wq32 = np.asarray(Wq, np.float32)
    Wv32 = np.asarray(Wv, np.float32)
    host_bias = []

    def pmajor(w):
        """[D, N] -> [P, KB, N] with [p, ko, n] = w[ko*128+p, n]"""
        return np.ascontiguousarray(
            w.reshape(KB, P, w.shape[1]).transpose(1, 0, 2))

    for c in range(NCORES):
        b, g = divmod(c, G)
        cols = slice(g * GC, (g + 1) * GC)
        wq_c = Wqkv[:, 0 * D:1 * D][:, cols].astype(np.float32)
        wk_c = Wqkv[:, 1 * D:2 * D][:, cols]
        wv_c = Wqkv[:, 2 * D:3 * D][:, cols]
        # fold the per-group query projection + attention scale on host:
        wqs = Wq32[g] * SCALE
        wq_fold = (wq_c.reshape(D, HPG, HD) @ wqs[None]).reshape(D, GC)
        wqk = np.concatenate([wq_fold.astype(bf), np.asarray(wk_c, bf)],
                             axis=1)
        bq1 = bqkv[0 * D:1 * D][cols].astype(np.float32)
        bk1 = bqkv[1 * D:2 * D][cols].astype(np.float32)
        bv1 = bqkv[2 * D:3 * D][cols].astype(np.float32)
        bq2 = (bq1.reshape(HPG, HD) @ wqs
               + np.asarray(bq, np.float32)[g] * SCALE).reshape(GC)
        b1cat = np.concatenate([bq2, bk1]).astype(np.float32)
        host_bias.append(
            (bv1.reshape(HPG, HD) @ Wv32[g]
             + np.asarray(bv, np.float32)[g][None, :]).reshape(GC))
        in_maps.append({
            "xt": x_bf[b],
            "wqk": pmajor(wqk),
            "wv1": pmajor(np.asarray(wv_c, bf)),
            "b1": np.ascontiguousarray(b1cat.reshape(8, P).T),
            "wv": np.ascontiguousarray(Wv32[g].astype(bf)),
            "onesc": np.ones((P, 1), bf),
            "sel": sel_np,
        })

    res = run_bass_kernel_spmd(nc, in_maps, list(range(NCORES)),
                               **_CACHE.get("run_kwargs", {}))
    _CACHE["last_results"] = res

    out = np.empty((B, S, D), np.float32)
    for c in range(NCORES):
        b, g = divmod(c, G)
        o = np.asarray(res.results[c]["out"], np.float32)  # [GC,S] out^T
        den = res.results[c]["den"]        # [HPG, S]
        o = o / np.repeat(den, HD, axis=0)  # normalize rows h*128+e by den[h]
        o = o + host_bias[c][:, None]
        out[b, :, g * GC:(g + 1) * GC] = o.T
    return out


# revision 35
# speedup vs baseline: 1.3503x; 1.0023x over previous
"""GQA dense-transformer kernel for 8 Trainium2 NeuronCores.

Problem (hardcoded): B=2, S=2048, D=2048, kv_heads=16, groups G=4, HPG=4,
HD=128.  reference:
    qkv = x @ Wqkv + bqkv ; q,k,v = split(qkv)
    q = einsum('bsghd,gde->bsghe', q, Wq) + bq   (per-group shared proj)
    v = einsum('bsghd,gde->bsghe', v, Wv) + bv
    scores = einsum('bqghd,bkghd->bghqk', q, k) / sqrt(HD)
    attn = softmax(scores) * attn_mask           (mask == ones at grading)
    out = einsum('bghqk,bkghd->bqghd', attn, v)  -> [B,S,D]

Sharding: core c = b*4 + g handles (batch b, group g): it computes the
512 output columns [g*512,(g+1)*512) of out[b].

Per-core device program (bf16 matmuls, fp32 PSUM):
  The per-group query projection Wq (and the attention scale) is folded
  into Wqkv on the HOST: W_q2'[:,hs] = Wqkv_q[:,hs] @ (Wq[g]*SCALE), so
  the device computes q2^T and k^T directly in one projection pass.
  All inputs are RELAID OUT partition-major on the host so every DMA
  descriptor covers a 4-16KB contiguous run (the dominant startup cost
  is per-descriptor overhead, not bytes).
  phase 1: chunk 0 (s in [0,512)) runs K-OUTER in groups of 4 k-blocks
           with 8 live PSUM banks (m=0..7 = q2',k), so the PE starts
           after ~1.5MB of DMA and then tracks the DMA stream; v
           (m=8..11) + chunks 1..3 run m-outer.  v2 natural blocks via
           lhsT=v1^T slices @ wv.  q2/k evacuate on ACT (bias add),
           v1/v2 on DVE.
  phase 2: software-pipelined across heads: for each j, scores(h) j
           (PE->PSUM, exp on ACT) is interleaved with PV(h-1) j
           (j-MAJOR accumulation into 4 simultaneous po banks), so the
           PE always has dense work while ACT drains the exps.
           Denominators per head via concurrent M=1 ones-matmuls in 4
           PE col-groups + one fp32r selector matmul (psum banks shared
           with po via tag rotation).  Output is UNNORMALIZED out^T +
           denominators; softmax division + v-path bias happen on host.
"""
import sys
import numpy as np

sys.path.insert(0, "/opt/trn_rl_repo")
import ml_dtypes  # noqa: E402

B, S, D = 2, 2048, 2048
G, HPG, HD = 4, 4, 128
GC = HPG * HD            # 512 columns per group
SCALE = HD ** -0.5
P = 128
KB = D // P              # 16 contraction blocks
SB = S // P              # 16 sk blocks
NCORES = 8

_CACHE: dict = {}


def _build_program():
    import concourse.tile_sem_assignment as tsa
    # Walrus caps sync waits per instruction; _split_excess_waits breaks
    # any multi-wait compute instruction into standalone EventSemaphore
    # CTRLs on the same engine.  Keep the default 8 HWDGE semaphores so
    # DMA-completion waits stay fine-grained (a single shared semaphore
    # makes every consumer wait on the cumulative count of all earlier
    # DMAs, which serializes the startup loads).
    tsa.NUM_HWDGE_SEMS = 8

    import concourse.bass as bass
    import concourse.tile as tile
    from concourse import mybir
    from contextlib import ExitStack

    bf16 = mybir.dt.bfloat16
    f32 = mybir.dt.float32

    nc = bass.Bass(trn_type="TRN2")
    SCH = 512                 # s-chunk width for projection phase
    NCH = S // SCH            # 4 chunks
    QCH = 1024                # sq chunk width for scores/exp
    # partition-major input layouts (see kernel() for the host side)
    xt_d = nc.dram_tensor("xt", [NCH, P, KB, SCH], bf16, kind="ExternalInput")
    wqk_d = nc.dram_tensor("wqk", [P, KB, 2 * GC], bf16, kind="ExternalInput")
    wv1_d = nc.dram_tensor("wv1", [P, KB, GC], bf16, kind="ExternalInput")
    b1_d = nc.dram_tensor("b1", [P, 8], f32, kind="ExternalInput")
    wv_d = nc.dram_tensor("wv", [HD, HD], bf16, kind="ExternalInput")
    onesc_d = nc.dram_tensor("onesc", [P, 1], bf16, kind="ExternalInput")
    sel_d = nc.dram_tensor("sel", [P, 1], f32, kind="ExternalInput")
    out_d = nc.dram_tensor("out", [GC, S], bf16, kind="ExternalOutput")
    den_d = nc.dram_tensor("den", [HPG, S], f32, kind="ExternalOutput")

    Exp = mybir.ActivationFunctionType.Exp
    Ident = mybir.ActivationFunctionType.Identity

    with tile.TileContext(nc) as tc:
        with ExitStack() as octx:
            # ---- persistent tiles ----
            persist = octx.enter_context(tc.tile_pool(name="persist", bufs=1))
            k_sb = persist.tile([P, HPG, S], bf16)       # k^T per head
            q2_sb = persist.tile([P, HPG, S], bf16)      # q2^T per head
            v2_sb = persist.tile([P, HPG, SB, HD], bf16)  # v2 natural blocks
            wv_sb = persist.tile([HD, HD], bf16)
            b1_sb = persist.tile([P, 8], f32)
            ones_sb = persist.tile([P, 1], bf16)
            sel_f = persist.tile([P, 1], f32)
            sel_sb = persist.tile([P, 1], mybir.dt.float32r)
            nc.sync.dma_start(wv_sb[:], wv_d[:])
            nc.sync.dma_start(b1_sb[:], b1_d[:])
            nc.sync.dma_start(ones_sb[:], onesc_d[:])
            nc.sync.dma_start(sel_f[:], sel_d[:])
            nc.vector.tensor_copy(sel_sb[:], sel_f[:])

            # ---------------- phase 1: projections ----------------
            with ExitStack() as ctx:
                wpool = ctx.enter_context(tc.tile_pool(name="w1", bufs=1))
                xpool = ctx.enter_context(tc.tile_pool(name="xT", bufs=3))
                tpool = ctx.enter_context(tc.tile_pool(name="tmp", bufs=4))
                wqk_sb = wpool.tile([P, KB, 2 * GC], bf16)
                wv1_sb = wpool.tile([P, KB, GC], bf16)
                xT0 = wpool.tile([P, KB, SCH], bf16)

                def wslice(m, k):
                    if m < 8:
                        return wqk_sb[:, k, m * P:(m + 1) * P]
                    return wv1_sb[:, k, (m - 8) * P:(m - 7) * P]

                # stage A: chunk 0, m=0..7 (q2', k), K-OUTER in groups of
                # 4 k-blocks with 8 live PSUM banks.  Each group's DMA is
                # one 128-descriptor transfer with 4-8KB contiguous runs.
                with ExitStack() as actx:
                    p0 = actx.enter_context(
                        tc.tile_pool(name="p0", bufs=1, space="PSUM"))
                    psA = [p0.tile([P, SCH], f32, tag=f"m{m}", name=f"psA{m}")
                           for m in range(8)]
                    # warm the PE during the initial DMA window: ~28 dummy
                    # matmuls on a memset scratch tile keep the HAM clock
                    # gate at 8/8 so the first real matmuls run at 2.4GHz.
                    ws = tpool.tile([P, SCH], bf16, tag="ws", name="ws")
                    nc.any.memset(ws[:], 0.0)
                    for _ in range(28):
                        nc.tensor.matmul(psA[0][:], ws[:, 0:P], ws[:],
                                         start=True, stop=True)
                    # first k-groups are small so the first matmul waits on
                    # ~0.4MB, not 1.5MB (single-DMA throughput ~40-60GB/s)
                    kgroups = [(0, 1), (1, 2), (2, 4), (4, 8), (8, 12),
                               (12, 16)]
                    for lo, hi in kgroups:
                        ks = slice(lo, hi)
                        nc.sync.dma_start(wqk_sb[:, ks], wqk_d[:, ks])
                        nc.sync.dma_start(xT0[:, ks], xt_d[0, :, ks])
                    nc.sync.dma_start(wv1_sb[:], wv1_d[:])
                    for k in range(KB):
                        for m in range(8):
                            nc.tensor.matmul(
                                psA[m][:], wslice(m, k), xT0[:, k],
                                start=(k == 0), stop=(k == KB - 1))
                    for m in range(8):
                        dst = q2_sb if m < 4 else k_sb
                        nc.scalar.activation(
                            dst[:, m % 4, 0:SCH], psA[m][:],
                            Ident, bias=b1_sb[:, m:m + 1])

                # stage B: v for chunk 0, then all m for chunks 1..3
                with ExitStack() as bctx:
                    pp = bctx.enter_context(
                        tc.tile_pool(name="pp", bufs=3, space="PSUM"))
                    pv = bctx.enter_context(
                        tc.tile_pool(name="pv", bufs=2, space="PSUM"))

                    def vblock(ps, h, c):
                        """v1 psum chunk -> v1 tmp -> v2 natural blocks."""
                        v1 = tpool.tile([P, SCH], bf16, tag="v1")
                        nc.vector.tensor_copy(v1[:], ps[:])
                        for sb in range(SCH // P):
                            ps3 = pv.tile([P, HD], f32)
                            nc.tensor.matmul(
                                ps3[:], v1[:, sb * P:(sb + 1) * P],
                                wv_sb[:], start=True, stop=True)
                            nc.vector.tensor_copy(
                                v2_sb[:, h, c * (SCH // P) + sb, :], ps3[:])

                    for m in range(8, 12):
                        ps = pp.tile([P, SCH], f32)
                        for k in range(KB):
                            nc.tensor.matmul(
                                ps[:], wslice(m, k), xT0[:, k],
                                start=(k == 0), stop=(k == KB - 1))
                        vblock(ps, m - 8, 0)

                    for c in range(1, NCH):
                        xT = xpool.tile([P, KB, SCH], bf16, tag="xT")
                        # 4 k-range pieces on separate queues: a single 2MB
                        # DMA (~50us on one engine) would pace the chunk.
                        for g4 in range(4):
                            ks = slice(4 * g4, 4 * g4 + 4)
                            nc.sync.dma_start(xT[:, ks], xt_d[c, :, ks])
                        for m in range(12):
                            ps = pp.tile([P, SCH], f32)
                            for k in range(KB):
                                nc.tensor.matmul(
                                    ps[:], wslice(m, k), xT[:, k],
                                    start=(k == 0), stop=(k == KB - 1))
                            if m < 8:
                                dst = q2_sb if m < 4 else k_sb
                                nc.scalar.activation(
                                    dst[:, m % 4, c * SCH:(c + 1) * SCH],
                                    ps[:], Ident, bias=b1_sb[:, m:m + 1])
                            else:
                                vblock(ps, m - 8, c)

            # ---------------- phase 2: attention ----------------
            # software pipeline: per j, scores(h) j interleaves with the
            # j-MAJOR PV of the SAME head at lag 1 (PV j-1 runs while
            # EXP j drains), so the PE tracks ACT with no dense-PV tail.
            with ExitStack() as ctx:
                ppool = ctx.enter_context(tc.tile_pool(name="P", bufs=24))
                opool = ctx.enter_context(tc.tile_pool(name="osb", bufs=4))
                dpool = ctx.enter_context(tc.tile_pool(name="dsb", bufs=4))
                sps = ctx.enter_context(
                    tc.tile_pool(name="sps", bufs=2, space="PSUM"))
                ops = ctx.enter_context(
                    tc.tile_pool(name="ops", bufs=1, space="PSUM"))

                def scores_j(h, j, Pj):
                    for qc in range(S // QCH):
                        ss = sps.tile([P, QCH], f32, name="ss")
                        for half in range(QCH // 512):
                            off = qc * QCH + half * 512
                            nc.tensor.matmul(
                                ss[:, half * 512:(half + 1) * 512],
                                k_sb[:, h, j * P:(j + 1) * P],
                                q2_sb[:, h, off:off + 512],
                                start=True, stop=True)
                        nc.scalar.activation(
                            Pj[:, qc * QCH:(qc + 1) * QCH], ss[:], Exp)

                def pv_den(h, Ph, po):
                    """emit PV epilogue + denominators for head h."""
                    for qc in range(4):
                        sl = slice(qc * 512, (qc + 1) * 512)
                        osb = opool.tile([P, 512], bf16, tag="o", name="osb")
                        nc.vector.tensor_copy(osb[:], po[qc][:])
                        nc.sync.dma_start(out_d[h * P:(h + 1) * P, sl], osb[:])
                        # denominators: 4 concurrent M=1 ones-matmuls in 4
                        # col-groups x 4 rounds, then fp32r selector matmul.
                        pd = ops.tile([P, 512], f32, tag=f"po{qc}", name="pd")
                        for r in range(4):
                            for jj in range(4):
                                j = r * 4 + jj
                                nc.tensor.matmul(
                                    pd[32 * jj:32 * jj + 1, :],
                                    ones_sb[:, 0:1], Ph[j][:, sl],
                                    start=(r == 0), stop=(r == 3),
                                    tile_position=(0, 32 * jj))
                        parts = dpool.tile([97, 512], mybir.dt.float32r,
                                           tag="dp", name="parts")
                        nc.vector.tensor_copy(parts[:], pd[0:97, :])
                        pd2 = ops.tile([P, 512], f32, tag=f"po{qc}",
                                       name="pd2")
                        nc.tensor.matmul(pd2[0:1, :], sel_sb[0:97, 0:1],
                                         parts[:], start=True, stop=True)
                        dsb = dpool.tile([1, 512], f32, tag="d", name="dsb")
                        nc.vector.tensor_copy(dsb[:], pd2[0:1, :])
                        nc.sync.dma_start(den_d[h:h + 1, sl], dsb[:])

                # Same-head PV at lag 3: PV(h) j runs while ACT drains
                # EXP(h) j+1..j+3, so the PE tracks ACT with only a ~3-j
                # PV tail per head.  pv_den(h-1) is emitted at j==3 of
                # head h (its exps finished a full block ago, so it never
                # stalls), IMMEDIATELY BEFORE po(h)'s allocation: the
                # shared po/pd psum-bank WAR chain then follows emission
                # order (osb(h-1) -> pd(h-1) -> pd2(h-1) -> po(h)), so no
                # deadlock.
                LAG = 3

                def pv_j(h, j, po, Ph):
                    for qc in range(4):
                        nc.tensor.matmul(
                            po[qc][:], v2_sb[:, h, j, :],
                            Ph[j][:, qc * 512:(qc + 1) * 512],
                            start=(j == 0), stop=(j == SB - 1))

                prev = None
                for h in range(HPG):
                    Ph = []
                    po = None
                    for j in range(SB):
                        Pj = ppool.tile([P, S], bf16, tag="P", name="Pj")
                        scores_j(h, j, Pj)
                        Ph.append(Pj)
                        if j == LAG:
                            if prev is not None:
                                pv_den(prev[0], prev[1], prev[2])
                            po = [ops.tile([P, 512], f32, tag=f"po{qc}",
                                           name=f"po{qc}") for qc in range(4)]
                        if j >= LAG:
                            pv_j(h, j - LAG, po, Ph)
                    for j in range(SB - LAG, SB):
                        pv_j(h, j, po, Ph)
                    prev = (h, Ph, po)
                pv_den(prev[0], prev[1], prev[2])

    _split_excess_waits(nc, mybir)
    return nc


def _split_excess_waits(nc, mybir):
    """Each TPB instruction has ONE wait slot (NEURON_ISA_TPB_EVENTS); walrus
    refuses instructions with more sync waits.  Tile attaches the full
    vector-clock wait list to instructions, so split all but one wait out
    into standalone EventSemaphore (CTRL) instructions on the same engine,
    placed immediately before.  Semantics are identical: all waits must be
    satisfied before the instruction executes."""
    import copy
    template = None
    for blk in nc.m.functions[0].blocks:
        for inst in blk.instructions:
            if isinstance(inst, mybir.InstEventSemaphore):
                template = inst
                break
        if template is not None:
            break
    assert template is not None, "no EventSemaphore template found"
    uid = [0]
    for fn in nc.m.functions:
        for blk in fn.blocks:
            out = []
            for inst in blk.instructions:
                si = inst.sync_info
                if si is not None and len(si.on_wait) > 1:
                    waits = list(si.on_wait)
                    for w in waits[:-1]:
                        ev = copy.deepcopy(template)
                        ev.name = f"swsplit-{uid[0]}"
                        uid[0] += 1
                        ev.engine = inst.engine
                        ev.sync_info = mybir.SyncInfo(on_wait=[w], on_update=[])
                        out.append(ev)
                    si.on_wait = waits[-1:]
                    inst.sync_info = si
                out.append(inst)
            blk.instructions[:] = out
    return nc


def _numpy_fallback(x, attn_mask, Wqkv, bqkv, Wq, bq, Wv, bv):
    x = np.asarray(x, np.float32)
    qkv = x @ np.asarray(Wqkv, np.float32) + np.asarray(bqkv, np.float32)
    q, k, v = np.split(qkv, 3, axis=-1)
    q = q.reshape(B, S, G, HPG, HD)
    k = k.reshape(B, S, G, HPG, HD)
    v = v.reshape(B, S, G, HPG, HD)
    q = np.einsum('bsghd,gde->bsghe', q, np.asarray(Wq, np.float32)) \
        + np.asarray(bq, np.float32)[None, None, :, None, :]
    v = np.einsum('bsghd,gde->bsghe', v, np.asarray(Wv, np.float32)) \
        + np.asarray(bv, np.float32)[None, None, :, None, :]
    out = np.empty((B, S, G, HPG, HD), np.float32)
    for b in range(B):
        for g in range(G):
            for hh in range(HPG):
                s = (q[b, :, g, hh] @ k[b, :, g, hh].T) * SCALE
                s = s - s.max(axis=-1, keepdims=True)
                p = np.exp(s)
                p /= p.sum(axis=-1, keepdims=True)
                p = p * np.asarray(attn_mask, np.float32)
                out[b, :, g, hh] = p @ v[b, :, g, hh]
    return out.reshape(B, S, D)


def kernel(x, attn_mask, Wqkv, bqkv, Wq, bq, Wv, bv):
    x = np.asarray(x)
    attn_mask = np.asarray(attn_mask)
    Wqkv = np.asarray(Wqkv)
    bqkv = np.asarray(bqkv)
    Wq = np.asarray(Wq)
    bq = np.asarray(bq)
    Wv = np.asarray(Wv)
    bv = np.asarray(bv)

    if not np.all(attn_mask == 1.0):
        # general (non-ones) post-softmax mask: correct but slow host path
        return _numpy_fallback(x, attn_mask, Wqkv, bqkv, Wq, bq, Wv, bv)

    if "nc" not in _CACHE:
        _CACHE["nc"] = _build_program()
    nc = _CACHE["nc"]
    from concourse.bass_utils import run_bass_kernel_spmd

    bf = ml_dtypes.bfloat16
    sel_np = np.zeros((P, 1), np.float32)
    sel_np[0::32] = 1.0
    in_maps = []
    # xt layout [chunk, p, ko, s']: xt[c,p,ko,s'] = x[b][c*512+s', ko*128+p]
    x_bf = []
    for b in range(B):
        xT = np.asarray(x[b], np.float32).T.astype(bf)      # [D, S]
        x_bf.append(np.ascontiguousarray(
            xT.reshape(KB, P, NCORES // 2, 512).transpose(2, 1, 0, 3)))
    Wq32 = np.asarray(Wq, np.float32)
    Wv32 = np.asarray(Wv, np.float32)
    host_bias = []

    def pmajor(w):
        """[D, N] -> [P, KB, N] with [p, ko, n] = w[ko*128+p, n]"""
        return np.ascontiguousarray(
            w.reshape(KB, P, w.shape[1]).transpose(1, 0, 2))

    for c in range(NCORES):
        b, g = divmod(c, G)
        cols = slice(g * GC, (g + 1) * GC)
        wq_c = Wqkv[:, 0 * D:1 * D][:, cols].astype(np.float32)
        wk_c = Wqkv[:, 1 * D:2 * D][:, cols]
        wv_c = Wqkv[:, 2 * D:3 * D][:, cols]
        # fold the per-group query projection + attention scale on host:
        wqs = Wq32[g] * SCALE
        wq_fold = (wq_c.reshape(D, HPG, HD) @ wqs[None]).reshape(D, GC)
        wqk = np.concatenate([wq_fold.astype(bf), np.asarray(wk_c, bf)],
                             axis=1)
        bq1 = bqkv[0 * D:1 * D][cols].astype(np.float32)
        bk1 = bqkv[1 * D:2 * D][cols].astype(np.float32)
        bv1 = bqkv[2 * D:3 * D][cols].astype(np.float32)
        bq2 = (bq1.reshape(HPG, HD) @ wqs
               + np.asarray(bq, np.float32)[g] * SCALE).reshape(GC)
        b1cat = np.concatenate([bq2, bk1]).astype(np.float32)
        host_bias.append(
            (bv1.reshape(HPG, HD) @ Wv32[g]
             + np.asarray(bv, np.float32)[g][None, :]).reshape(GC))
        in_maps.append({
            "xt": x_bf[b],
            "wqk": pmajor(wqk),
            "wv1": pmajor(np.asarray(wv_c, bf)),
            "b1": np.ascontiguousarray(b1cat.reshape(8, P).T),
            "wv": np.ascontiguousarray(Wv32[g].astype(bf)),
            "onesc": np.ones((P, 1), bf),
            "sel": sel_np,
        })

    res = run_bass_kernel_spmd(nc, in_maps, list(range(NCORES)),
                               **_CACHE.get("run_kwargs", {}))
    _CACHE["last_results"] = res

    out = np.empty((B, S, D), np.float32)
    for c in range(NCORES):
        b, g = divmod(c, G)
        o = np.asarray(res.results[c]["out"], np.float32)  # [GC,S] out^T
        den = res.results[c]["den"]        # [HPG, S]
        o = o / np.repeat(den, HD, axis=0)  # normalize rows h*128+e by den[h]
        o = o + host_bias[c][:, None]
        out[b, :, g * GC:(g + 1) * GC] = o.T
    return out
